# revision 1
# baseline (speedup 1.0000x reference)
"""DIFFormerConv (simple linear attention + dense GCN) on 8 trn2 NeuronCores.

Sharding: nodes N=4096 split 8 ways (S=512 per core). Each core computes
q/k/v for its node shard, partial kvs/ks_sum/vsum (AllReduce), vmean
(AllGather, bf16), the attention output rows for its shard, and the GCN
rows for its shard (adj^T column shard, bf16 matmul).

Layouts chosen so no PE transposes are needed:
  q:   [hd, s]  (heads*dim on partitions)   -- lhsT = W^T chunks
  k,v: [s, hd]  (transposed projection)     -- lhsT = x chunks
  gcn: [(b,d), n] directly                  -- lhsT = vmean[m,(b,d)], rhs = adjT[m,n]
Host prep: adjT = adj.T + I (bf16), rrs = 0.25/(rowsum+1), W transposes.
"""

import sys

sys.path.insert(0, "/opt/trn_rl_repo")

import numpy as np
import ml_dtypes

from concourse import bass, bacc, tile, mybir
from concourse.bass_utils import run_bass_kernel_spmd

B, C, N, H, D = 8, 256, 4096, 4, 64
NCORES = 8
S = N // NCORES          # 512 nodes per core
HD = H * D               # 256
F32 = mybir.dt.float32
F32R = mybir.dt.float32r
BF16 = mybir.dt.bfloat16
AX = mybir.AxisListType.X
ALU = mybir.AluOpType
ACTF = mybir.ActivationFunctionType
RG = [list(range(NCORES))]

_CACHE = {}
DEBUG_DUMPS = False


def _indicators():
    i4a = np.zeros((128, 4), np.float32)
    i4b = np.zeros((128, 4), np.float32)
    for p in range(128):
        i4a[p, p // 64] = 1.0
        i4b[p, 2 + p // 64] = 1.0
    ibc0 = np.zeros((4, 128), np.float32)
    ibc1 = np.zeros((4, 128), np.float32)
    for p in range(128):
        ibc0[p // 64, p] = 1.0
        ibc1[2 + p // 64, p] = 1.0
    return i4a, i4b, ibc0, ibc1


def _build():
    nc = bacc.Bacc("TRN2", target_bir_lowering=False, debug=False,
                   num_devices=NCORES)

    xq = nc.dram_tensor("xq", [B, 2, 128, S], F32R, kind="ExternalInput")
    xs = nc.dram_tensor("xs", [B, 2, 128, S], F32R, kind="ExternalInput")
    adjt = nc.dram_tensor("adjt", [32, 128, S], BF16, kind="ExternalInput")
    rrs = nc.dram_tensor("rrs", [1, S], F32R, kind="ExternalInput")
    wqt = nc.dram_tensor("wqt", [2, 128, HD], F32R, kind="ExternalInput")
    wkt = nc.dram_tensor("wkt", [2, 128, HD], F32R, kind="ExternalInput")
    wvt = nc.dram_tensor("wvt", [2, 128, HD], F32R, kind="ExternalInput")
    bqr = nc.dram_tensor("bqr", [1, HD], F32R, kind="ExternalInput")
    bkr = nc.dram_tensor("bkr", [1, HD], F32R, kind="ExternalInput")
    bvr = nc.dram_tensor("bvr", [1, HD], F32R, kind="ExternalInput")
    out = nc.dram_tensor("out", [B, D, S], F32, kind="ExternalOutput")
    if DEBUG_DUMPS:
        dbg_ar = nc.dram_tensor("dbg_ar", [B, 2, 132, D], F32,
                                kind="ExternalOutput")
        dbg_vm = nc.dram_tensor("dbg_vm", [NCORES, S, B, D], BF16,
                                kind="ExternalOutput")

    i4a_d = nc.dram_tensor("i4a_in", [128, 4], F32R, kind="ExternalInput")
    i4b_d = nc.dram_tensor("i4b_in", [128, 4], F32R, kind="ExternalInput")
    ibc0_d = nc.dram_tensor("ibc0_in", [4, 128], F32R, kind="ExternalInput")
    ibc1_d = nc.dram_tensor("ibc1_in", [4, 128], F32R, kind="ExternalInput")
    ones_r_d = nc.dram_tensor("ones_r", [1, S], F32R, kind="ExternalInput")
    ones_c_d = nc.dram_tensor("ones_c", [128, 1], F32R, kind="ExternalInput")

    def r(ap):
        return ap

    with nc.allow_low_precision(reason="float32r rounding intentional"), \
            tile.TileContext(nc) as tc:
        with (
            tc.tile_pool(name="pers", bufs=1) as pp,
            tc.tile_pool(name="work", bufs=3) as wk,
            tc.tile_pool(name="dram", bufs=1, space="DRAM") as dp,
        ):
            # DRAM internal buffers for collectives
            vm_loc = dp.tile([S, B, D], BF16, tag="vm_loc", name="vm_loc")
            vm_all = dp.tile([NCORES, S, B, D], BF16, tag="vm_all", name="vm_all", addr_space="Shared")
            ar_in = dp.tile([B, 2, 132, D], F32, tag="ar_in", name="ar_in")
            ar_out = dp.tile([B, 2, 132, D], F32, tag="ar_out", name="ar_out", addr_space="Shared")

            # ---- constants ----
            wq_t = [pp.tile([128, HD], F32R, tag=f"wq{c}", name=f"wq{c}") for c in range(2)]
            wk_t = [pp.tile([128, HD], F32R, tag=f"wk{c}", name=f"wk{c}") for c in range(2)]
            wv_t = [pp.tile([128, HD], F32R, tag=f"wv{c}", name=f"wv{c}") for c in range(2)]
            for c in range(2):
                nc.sync.dma_start(out=wq_t[c][:], in_=wqt[c])
                nc.sync.dma_start(out=wk_t[c][:], in_=wkt[c])
                nc.sync.dma_start(out=wv_t[c][:], in_=wvt[c])
            bq_row = pp.tile([1, HD], F32R, tag="bqrow")
            bk_row = pp.tile([1, HD], F32R, tag="bkrow")
            bv_row = pp.tile([1, HD], F32R, tag="bvrow")
            nc.sync.dma_start(out=bq_row[:], in_=bqr[:])
            nc.sync.dma_start(out=bk_row[:], in_=bkr[:])
            nc.sync.dma_start(out=bv_row[:], in_=bvr[:])
            i4a = pp.tile([128, 4], F32R, tag="i4a")
            i4b = pp.tile([128, 4], F32R, tag="i4b")
            ibc0 = pp.tile([4, 128], F32R, tag="ibc0")
            ibc1 = pp.tile([4, 128], F32R, tag="ibc1")
            nc.sync.dma_start(out=i4a[:], in_=i4a_d[:])
            nc.sync.dma_start(out=i4b[:], in_=i4b_d[:])
            nc.sync.dma_start(out=ibc0[:], in_=ibc0_d[:])
            nc.sync.dma_start(out=ibc1[:], in_=ibc1_d[:])
            ones_row = pp.tile([1, S], F32R, tag="ones_row")
            ones_col = pp.tile([128, 1], F32R, tag="ones_col")
            nc.sync.dma_start(out=ones_row[:], in_=ones_r_d[:])
            nc.sync.dma_start(out=ones_col[:], in_=ones_c_d[:])
            rrs_row = pp.tile([1, S], F32R, tag="rrs_row")
            nc.sync.dma_start(out=rrs_row[:], in_=rrs[:])

            # persistent per-batch SBUF tensors
            q_sb = [[pp.tile([128, S], F32R, tag=f"q{b}_{h}", name=f"q{b}_{h}") for h in range(2)]
                    for b in range(B)]
            kt_sb = [[pp.tile([128, HD], F32R, tag=f"kt{b}_{s}", name=f"kt{b}_{s}") for s in range(4)]
                     for b in range(B)]
            vt_sb = [[pp.tile([128, HD], F32R, tag=f"vt{b}_{s}", name=f"vt{b}_{s}") for s in range(4)]
                     for b in range(B)]
            rq_sb = [pp.tile([4, S], F32, tag=f"rq{b}", name=f"rq{b}") for b in range(B)]
            attn_sb = [pp.tile([128, S], F32, tag=f"at{p}", name=f"at{p}")
                       for p in range(4)]
            rrs_bc = pp.tile([128, S], F32, tag="rrs_bc")

            with tc.tile_pool(name="psA", bufs=1, space="PSUM") as psA:
                # broadcast rrs row to all 128 partitions (K=1 matmul)
                pbc0 = psA.tile([128, S], F32, tag="pq")
                nc.tensor.matmul(pbc0[:], lhsT=r(ones_row[:, 0:128]),
                                 rhs=r(rrs_row[:]), start=True, stop=True)
                nc.scalar.activation(rrs_bc[:], pbc0[:], ACTF.Copy)

                # =================== phase 1: per-batch local ===================
                for b in range(B):
                    xs0 = wk.tile([128, S], F32R, tag="xs0", bufs=2)
                    xs1 = wk.tile([128, S], F32R, tag="xs1", bufs=2)
                    nc.sync.dma_start(out=xs0[:], in_=xs[b, 0])
                    nc.sync.dma_start(out=xs1[:], in_=xs[b, 1])

                    kvs_ps0 = psA.tile([128, HD], F32, tag="kvs0")
                    kvs_ps1 = psA.tile([128, HD], F32, tag="kvs1")
                    ks_ps = psA.tile([1, HD], F32, tag="ksps")
                    vs_ps = psA.tile([1, HD], F32, tag="vsps")

                    for sb_i in range(4):
                        sl = slice(sb_i * 128, (sb_i + 1) * 128)
                        # k^T and v^T projections: out[s, hd]
                        pk = psA.tile([128, HD], F32, tag="pk")
                        pv = psA.tile([128, HD], F32, tag="pv")
                        for (ps, wt, brow) in ((pk, wk_t, bk_row),
                                               (pv, wv_t, bv_row)):
                            nc.tensor.matmul(ps[:], lhsT=r(xs0[:, sl]),
                                             rhs=r(wt[0][:]), start=True,
                                             stop=False)
                            nc.tensor.matmul(ps[:], lhsT=r(xs1[:, sl]),
                                             rhs=r(wt[1][:]), start=False,
                                             stop=False)
                            nc.tensor.matmul(ps[:], lhsT=r(ones_row[:, 0:128]),
                                             rhs=r(brow[:]), start=False,
                                             stop=True)
                        # v^T evac
                        nc.scalar.activation(vt_sb[b][sb_i][:], pv[:], ACTF.Copy)
                        # vmean (sum over heads; /4 folded into rrs) -> bf16
                        vm_t = wk.tile([128, D], BF16, tag="vmt")
                        with nc.allow_low_precision(reason="vmean bf16 is ok"):
                            nc.vector.reduce_sum(
                                vm_t[:], pv[:].rearrange("p (h d) -> p d h",
                                                         h=H),
                                axis=AX)
                        nc.sync.dma_start(out=vm_loc[sl, b, :], in_=vm_t[:])
                        # kn = k / ||k||  (per head, free-dim blocks of 64)
                        sq = wk.tile([128, HD], F32, tag="sq")
                        nc.scalar.activation(sq[:], pk[:], ACTF.Square)
                        ssk = wk.tile([128, H], F32, tag="ssk")
                        nc.vector.reduce_sum(
                            ssk[:], sq[:].rearrange("p (h d) -> p h d", h=H),
                            axis=AX)
                        snk = wk.tile([128, H], F32, tag="snk")
                        nc.scalar.activation(snk[:], ssk[:], ACTF.Sqrt)
                        rk = wk.tile([128, H], F32, tag="rk")
                        nc.vector.reciprocal(rk[:], snk[:])
                        for h in range(H):
                            dsl = slice(h * D, (h + 1) * D)
                            nc.vector.tensor_scalar_mul(
                                kt_sb[b][sb_i][:, dsl], pk[:, dsl],
                                rk[:, h:h + 1])

                    # kvs / ks_sum / vsum partials, one contiguous
                    # accumulation group per bank
                    for sb_i in range(4):
                        nc.tensor.matmul(kvs_ps0[:],
                                         lhsT=r(kt_sb[b][sb_i][:, 0:128]),
                                         rhs=r(vt_sb[b][sb_i][:]),
                                         start=(sb_i == 0), stop=(sb_i == 3))
                    for sb_i in range(4):
                        nc.tensor.matmul(kvs_ps1[:],
                                         lhsT=r(kt_sb[b][sb_i][:, 128:HD]),
                                         rhs=r(vt_sb[b][sb_i][:]),
                                         start=(sb_i == 0), stop=(sb_i == 3))
                    for sb_i in range(4):
                        nc.tensor.matmul(ks_ps[:], lhsT=r(ones_col[:]),
                                         rhs=r(kt_sb[b][sb_i][:]),
                                         start=(sb_i == 0), stop=(sb_i == 3))
                    for sb_i in range(4):
                        nc.tensor.matmul(vs_ps[:], lhsT=r(ones_col[:]),
                                         rhs=r(vt_sb[b][sb_i][:]),
                                         start=(sb_i == 0), stop=(sb_i == 3))

                    # evac kvs diag blocks (stacked [128,(h,m) x 64 d])
                    pk0 = wk.tile([128, D], F32, tag="arpk0")
                    pk1 = wk.tile([128, D], F32, tag="arpk1")
                    nc.scalar.activation(pk0[0:64, :], kvs_ps0[0:64, 0:64],
                                         ACTF.Copy)
                    nc.scalar.activation(pk0[64:128, :],
                                         kvs_ps0[64:128, 64:128], ACTF.Copy)
                    nc.scalar.activation(pk1[0:64, :], kvs_ps1[0:64, 128:192],
                                         ACTF.Copy)
                    nc.scalar.activation(pk1[64:128, :],
                                         kvs_ps1[64:128, 192:256], ACTF.Copy)
                    ksvs_sb = wk.tile([1, 2 * HD], F32, tag="ksvs_sb", bufs=2)
                    nc.scalar.activation(ksvs_sb[0:1, 0:HD], ks_ps[:],
                                         ACTF.Copy)
                    nc.scalar.activation(ksvs_sb[0:1, HD:2 * HD], vs_ps[:],
                                         ACTF.Copy)
                    nc.sync.dma_start(out=ar_in[b, 0, 0:128, :], in_=pk0[:])
                    nc.sync.dma_start(out=ar_in[b, 1, 0:128, :], in_=pk1[:])
                    nc.sync.dma_start(out=ar_in[b, 0, 128:130, :],
                                      in_=ksvs_sb[0:1, 0:128])
                    nc.sync.dma_start(out=ar_in[b, 1, 128:130, :],
                                      in_=ksvs_sb[0:1, 128:256])
                    nc.sync.dma_start(out=ar_in[b, 0, 130:132, :],
                                      in_=ksvs_sb[0:1, 256:384])
                    nc.sync.dma_start(out=ar_in[b, 1, 130:132, :],
                                      in_=ksvs_sb[0:1, 384:512])

                    # q projection: out[hd, s]
                    xq0 = wk.tile([128, S], F32R, tag="xs0", bufs=2)
                    xq1 = wk.tile([128, S], F32R, tag="xs1", bufs=2)
                    nc.sync.dma_start(out=xq0[:], in_=xq[b, 0])
                    nc.sync.dma_start(out=xq1[:], in_=xq[b, 1])
                    ss_ps = psA.tile([4, S], F32, tag="ss")
                    for h in range(2):
                        hsl = slice(h * 128, (h + 1) * 128)
                        pq = psA.tile([128, S], F32, tag="pq")
                        nc.tensor.matmul(pq[:], lhsT=r(wq_t[0][:, hsl]),
                                         rhs=r(xq0[:]), start=True, stop=False)
                        nc.tensor.matmul(pq[:], lhsT=r(wq_t[1][:, hsl]),
                                         rhs=r(xq1[:]), start=False, stop=False)
                        nc.tensor.matmul(pq[:], lhsT=r(bq_row[:, hsl]),
                                         rhs=r(ones_row[:]), start=False,
                                         stop=True)
                        nc.scalar.activation(q_sb[b][h][:], pq[:], ACTF.Copy)
                        qsq = wk.tile([128, S], F32R, tag="qsq", bufs=2)
                        nc.scalar.activation(qsq[:], pq[:], ACTF.Square)
                        nc.tensor.matmul(ss_ps[:],
                                         lhsT=r(i4a[:] if h == 0 else i4b[:]),
                                         rhs=r(qsq[:]), start=(h == 0),
                                         stop=(h == 1))
                    snq = wk.tile([4, S], F32, tag="snq", bufs=1)
                    nc.scalar.activation(snq[:], ss_ps[:], ACTF.Sqrt)
                    nc.vector.reciprocal(rq_sb[b][:], snq[:])

            # =================== collectives ===================
            nc.gpsimd.collective_compute(
                "AllGather", ALU.bypass, ins=[vm_loc.opt()],
                outs=[vm_all.opt()], replica_groups=RG)
            nc.gpsimd.collective_compute(
                "AllReduce", ALU.add, ins=[ar_in.opt()],
                outs=[ar_out.opt()], replica_groups=RG)

            # =================== phase 2: attention epilogue ===================
            with tc.tile_pool(name="psB", bufs=2, space="PSUM") as psB:
                for b in range(B):
                    kpk0f = wk.tile([128, D], F32, tag="kpk0f")
                    kpk1f = wk.tile([128, D], F32, tag="kpk1f")
                    nc.sync.dma_start(out=kpk0f[:], in_=ar_out[b, 0, 0:128, :])
                    nc.sync.dma_start(out=kpk1f[:], in_=ar_out[b, 1, 0:128, :])
                    kpk0 = wk.tile([128, D], F32R, tag="kpk0")
                    kpk1 = wk.tile([128, D], F32R, tag="kpk1")
                    nc.scalar.activation(kpk0[:], kpk0f[:], ACTF.Copy)
                    nc.scalar.activation(kpk1[:], kpk1f[:], ACTF.Copy)
                    ksp0f = wk.tile([128, 4], F32, tag="ksp0f")
                    ksp1f = wk.tile([128, 4], F32, tag="ksp1f")
                    nc.vector.memset(ksp0f[:], 0.0)
                    nc.vector.memset(ksp1f[:], 0.0)
                    nc.sync.dma_start(out=ksp0f[0:64, 0:1],
                                      in_=ar_out[b, 0, 128, :])
                    nc.sync.dma_start(out=ksp0f[64:128, 1:2],
                                      in_=ar_out[b, 0, 129, :])
                    nc.sync.dma_start(out=ksp1f[0:64, 2:3],
                                      in_=ar_out[b, 1, 128, :])
                    nc.sync.dma_start(out=ksp1f[64:128, 3:4],
                                      in_=ar_out[b, 1, 129, :])
                    ksp0 = wk.tile([128, 4], F32R, tag="ksp0")
                    ksp1 = wk.tile([128, 4], F32R, tag="ksp1")
                    nc.scalar.activation(ksp0[:], ksp0f[:], ACTF.Copy)
                    nc.scalar.activation(ksp1[:], ksp1f[:], ACTF.Copy)
                    vspf = wk.tile([4, D], F32, tag="vspf")
                    nc.sync.dma_start(out=vspf[0:2, :],
                                      in_=ar_out[b, 0, 130:132, :])
                    nc.sync.dma_start(out=vspf[2:4, :],
                                      in_=ar_out[b, 1, 130:132, :])
                    vsp = wk.tile([4, D], F32R, tag="vsp")
                    nc.scalar.activation(vsp[:], vspf[:], ACTF.Copy)

                    pden = psB.tile([4, S], F32, tag="pb")
                    nc.tensor.matmul(pden[:], lhsT=r(ksp0[:]),
                                     rhs=r(q_sb[b][0][:]), start=True,
                                     stop=False)
                    nc.tensor.matmul(pden[:], lhsT=r(ksp1[:]),
                                     rhs=r(q_sb[b][1][:]), start=False,
                                     stop=True)
                    t0 = wk.tile([4, S], F32, tag="t0", bufs=1)
                    nc.vector.tensor_mul(t0[:], pden[:], rq_sb[b][:])
                    t1 = wk.tile([4, S], F32, tag="t1", bufs=1)
                    nc.vector.tensor_scalar(t1[:], t0[:], 4.0, float(4 * N),
                                            op0=ALU.mult, op1=ALU.add)
                    rp = wk.tile([4, S], F32R, tag="rp", bufs=2)
                    nc.vector.reciprocal(rp[:], t1[:])  # 0.25/denom
                    cc = wk.tile([4, S], F32R, tag="cc", bufs=2)
                    nc.vector.tensor_mul(cc[:], rp[:].bitcast(F32), rq_sb[b][:])

                    pat = psB.tile([D, S], F32, tag="pat")
                    for h in range(2):
                        pbc = psB.tile([128, S], F32, tag="pb")
                        nc.tensor.matmul(pbc[:],
                                         lhsT=r(ibc0[:] if h == 0 else ibc1[:]),
                                         rhs=r(cc[:]), start=True, stop=True)
                        qs = wk.tile([128, S], F32R, tag="qs", bufs=2)
                        nc.vector.tensor_mul(qs[:], q_sb[b][h][:].bitcast(F32), pbc[:])
                        nc.tensor.matmul(pat[:],
                                         lhsT=r(kpk0[:] if h == 0 else kpk1[:]),
                                         rhs=r(qs[:]), start=(h == 0),
                                         stop=False)
                    nc.tensor.matmul(pat[:], lhsT=r(vsp[:]), rhs=r(rp[:]),
                                     start=False, stop=True)
                    nc.scalar.activation(
                        attn_sb[b // 2][(b % 2) * D:(b % 2 + 1) * D, :],
                        pat[:], ACTF.Copy)

                # =================== phase 3: GCN ===================
                with tc.tile_pool(name="psC", bufs=1, space="PSUM") as psC:
                    pg = [psC.tile([128, S], F32, tag=f"g{p}", name=f"g{p}") for p in range(4)]
                    for mc in range(32):
                        adj_t = wk.tile([128, S], BF16, tag="adj")
                        nc.sync.dma_start(out=adj_t[:], in_=adjt[mc])
                        for p in range(4):
                            vm_t = wk.tile([128, 128], BF16, tag="vml")
                            lc = mc % 4
                            nc.sync.dma_start(
                                out=vm_t[:],
                                in_=vm_all[mc // 4,
                                           lc * 128:(lc + 1) * 128,
                                           2 * p:2 * p + 2, :])
                            nc.tensor.matmul(pg[p][:], lhsT=vm_t[:],
                                             rhs=adj_t[:], start=(mc == 0),
                                             stop=(mc == 31))
                    for p in range(4):
                        gt = wk.tile([128, S], F32, tag="gt", bufs=2)
                        nc.vector.tensor_mul(gt[:], pg[p][:], rrs_bc[:])
                        ot = wk.tile([128, S], F32, tag="ot", bufs=2)
                        nc.vector.tensor_add(ot[:], gt[:], attn_sb[p][:])
                        nc.sync.dma_start(out=out[2 * p], in_=ot[0:D, :])
                        nc.sync.dma_start(out=out[2 * p + 1], in_=ot[D:128, :])
                    if DEBUG_DUMPS:
                        nc.sync.dma_start(out=dbg_ar[:], in_=ar_out[:])
                        nc.sync.dma_start(out=dbg_vm[:], in_=vm_all[:])
    nc.compile()
    return nc


def _prep_inputs(query_input, source_input, adj, Wq_w, Wq_b, Wk_w, Wk_b,
                 Wv_w, Wv_b):
    xq_np = np.asarray(query_input, dtype=np.float32)
    xs_np = np.asarray(source_input, dtype=np.float32)
    adj_np = np.asarray(adj, dtype=np.float32)

    adjT = np.ascontiguousarray(adj_np.T)
    np.fill_diagonal(adjT, adjT.diagonal() + 1.0)
    adjT_bf = adjT.astype(ml_dtypes.bfloat16)
    rrs_full = (0.25 / (adj_np.sum(axis=1) + 1.0)).astype(np.float32)

    wqt = np.ascontiguousarray(np.asarray(Wq_w, np.float32).T).reshape(2, 128, HD)
    wkt = np.ascontiguousarray(np.asarray(Wk_w, np.float32).T).reshape(2, 128, HD)
    wvt = np.ascontiguousarray(np.asarray(Wv_w, np.float32).T).reshape(2, 128, HD)
    bq = np.asarray(Wq_b, np.float32).reshape(1, HD)
    bk = np.asarray(Wk_b, np.float32).reshape(1, HD)
    bv = np.asarray(Wv_b, np.float32).reshape(1, HD)

    i4a, i4b, ibc0, ibc1 = _indicators()
    in_maps = []
    for i in range(NCORES):
        sl = slice(i * S, (i + 1) * S)
        in_maps.append({
            "xq": np.ascontiguousarray(xq_np[:, :, sl]).reshape(B, 2, 128, S),
            "xs": np.ascontiguousarray(xs_np[:, :, sl]).reshape(B, 2, 128, S),
            "adjt": np.ascontiguousarray(adjT_bf[:, sl]).reshape(32, 128, S),
            "rrs": np.ascontiguousarray(rrs_full[sl]).reshape(1, S),
            "wqt": wqt, "wkt": wkt, "wvt": wvt,
            "bqr": bq, "bkr": bk, "bvr": bv,
            "i4a_in": i4a, "i4b_in": i4b,
            "ibc0_in": ibc0, "ibc1_in": ibc1,
            "ones_r": np.ones((1, S), np.float32),
            "ones_c": np.ones((128, 1), np.float32),
        })
    return in_maps


def kernel(**inputs):
    if "nc" not in _CACHE:
        _CACHE["nc"] = _build()
    nc = _CACHE["nc"]
    in_maps = _prep_inputs(**inputs)
    res = run_bass_kernel_spmd(nc, in_maps, list(range(NCORES)))
    full = np.empty((B, D, N), np.float32)
    for i in range(NCORES):
        full[:, :, i * S:(i + 1) * S] = res.results[i]["out"]
    return full



# revision 10
# speedup vs baseline: 1.5150x; 1.5150x over previous
"""DIFFormerConv (simple linear attention + dense GCN) on 8 trn2 NeuronCores.

Sharding: nodes N=4096 split 8 ways (S=512 per core). Each core computes
q/k/v for its node shard, partial kvs/ks_sum/vsum (AllReduce), vmean
(AllGather, bf16), the attention output rows for its shard, and the GCN
rows for its shard (adj^T column shard, bf16 matmul).

All matmuls run in bf16 (full PE rate). Phase order maximizes
collective overlap: k/v/kvs/vmean for all batches -> AllReduce +
AllGather issued -> q projections run under the collectives -> attention
epilogue (batched denominators) -> GCN.

Engines can only address SBUF/PSUM at partition offsets 0/32/64, so all
per-batch [4,*] rows are produced via indicator matmuls accumulating
into batched [32,S]/[B,HD] PSUM tiles, and repacked to [4, B*S] layout
with small SBUF->SBUF DMAs where matmul operands need base partition 0.

Layouts (no PE transposes needed):
  q:   [hd, s]  (heads*dim on partitions)  -- lhsT = W^T chunks, bias via
                                              per-partition activation add
  k,v: [s, hd]  (transposed projection)    -- lhsT = x chunks, bias via
                                              K=1 ones matmul
  kvs AllReduce payload: [2, 128, B, 65] f32 (64 diag cols + ks column)
  gcn: [(b,d), n] directly                 -- lhsT = vmean[m,(b,d)],
                                              rhs = adjT[m,n], bf16
Host prep: adjT = adj.T + I (bf16), rrs = 0.25/(rowsum+1), W^T in bf16.
"""

import sys

sys.path.insert(0, "/opt/trn_rl_repo")

import numpy as np
import ml_dtypes

from concourse import bass, bacc, tile, mybir
from concourse.bass_utils import run_bass_kernel_spmd

B, C, N, H, D = 8, 256, 4096, 4, 64
NCORES = 8
S = N // NCORES          # 512 nodes per core
HD = H * D               # 256
F32 = mybir.dt.float32
F32R = mybir.dt.float32r
BF16 = mybir.dt.bfloat16
AX = mybir.AxisListType.X
ALU = mybir.AluOpType
ACTF = mybir.ActivationFunctionType
RG = [list(range(NCORES))]

_CACHE = {}


def _build():
    nc = bacc.Bacc("TRN2", target_bir_lowering=False, debug=False,
                   num_devices=NCORES)

    xq = nc.dram_tensor("xq", [B, 2, 128, S], BF16, kind="ExternalInput")
    xs = nc.dram_tensor("xs", [B, 2, 128, S], BF16, kind="ExternalInput")
    adjt = nc.dram_tensor("adjt", [32, 128, S], BF16, kind="ExternalInput")
    rrs = nc.dram_tensor("rrs", [1, S], F32R, kind="ExternalInput")
    wqt = nc.dram_tensor("wqt", [2, 128, HD], BF16, kind="ExternalInput")
    wkt = nc.dram_tensor("wkt", [2, 128, HD], BF16, kind="ExternalInput")
    wvt = nc.dram_tensor("wvt", [2, 128, HD], BF16, kind="ExternalInput")
    bqc = nc.dram_tensor("bqc", [128, 2], F32, kind="ExternalInput")
    bkr = nc.dram_tensor("bkr", [1, HD], BF16, kind="ExternalInput")
    bvr = nc.dram_tensor("bvr", [1, HD], BF16, kind="ExternalInput")
    out = nc.dram_tensor("out", [B, D, S], F32, kind="ExternalOutput")
    import os
    DBG = bool(os.environ.get("KDBG"))
    if DBG:
        dbg1 = nc.dram_tensor("dbg1", [2, 128, B, 65], F32,
                              kind="ExternalOutput")

    # indicator tensors (see _prep_inputs)
    iqa_d = nc.dram_tensor("iqa_in", [128, B * 32], BF16, kind="ExternalInput")
    iqb_d = nc.dram_tensor("iqb_in", [128, B * 32], BF16, kind="ExternalInput")
    iv8_d = nc.dram_tensor("iv8_in", [128, B * B], BF16, kind="ExternalInput")
    ibc0_d = nc.dram_tensor("ibc0_in", [4, 128], BF16, kind="ExternalInput")
    ibc1_d = nc.dram_tensor("ibc1_in", [4, 128], BF16, kind="ExternalInput")
    ones_rb_d = nc.dram_tensor("ones_rb", [1, 128], BF16, kind="ExternalInput")
    ones_rf_d = nc.dram_tensor("ones_rf", [1, 128], F32R, kind="ExternalInput")
    ones_c_d = nc.dram_tensor("ones_c", [128, 1], BF16, kind="ExternalInput")

    with nc.allow_low_precision(reason="bf16 matmul pipeline intentional"), \
            tile.TileContext(nc) as tc:
        with (
            tc.tile_pool(name="pers", bufs=1) as pp,
            tc.tile_pool(name="work", bufs=3) as wk,
            tc.tile_pool(name="dram", bufs=1, space="DRAM") as dp,
        ):
            # DRAM internal buffers for collectives
            vm_loc = dp.tile([S, B, D], BF16, tag="vm_loc", name="vm_loc")
            vm_all = dp.tile([NCORES, S, B, D], BF16, tag="vm_all",
                             name="vm_all", addr_space="Shared")
            ar1_in = dp.tile([2, 128, B, 65], F32, tag="ar1_in",
                             name="ar1_in")
            ar1_out = dp.tile([2, 128, B, 65], F32, tag="ar1_out",
                              name="ar1_out", addr_space="Shared")
            ar2_in = dp.tile([B, HD], F32, tag="ar2_in", name="ar2_in")
            ar2_out = dp.tile([B, HD], F32, tag="ar2_out", name="ar2_out",
                              addr_space="Shared")

            # ---- constants ----
            wq_t = [pp.tile([128, HD], BF16, tag=f"wq{c}", name=f"wq{c}")
                    for c in range(2)]
            wk_t = [pp.tile([128, HD], BF16, tag=f"wk{c}", name=f"wk{c}")
                    for c in range(2)]
            wv_t = [pp.tile([128, HD], BF16, tag=f"wv{c}", name=f"wv{c}")
                    for c in range(2)]
            for c in range(2):
                nc.sync.dma_start(out=wq_t[c][:], in_=wqt[c])
                nc.sync.dma_start(out=wk_t[c][:], in_=wkt[c])
                nc.sync.dma_start(out=wv_t[c][:], in_=wvt[c])
            bq_sb = pp.tile([128, 2], F32, tag="bqsb")
            bk_sb = pp.tile([1, HD], BF16, tag="bksb")
            bv_sb = pp.tile([1, HD], BF16, tag="bvsb")
            nc.sync.dma_start(out=bq_sb[:], in_=bqc[:])
            nc.sync.dma_start(out=bk_sb[:], in_=bkr[:])
            nc.sync.dma_start(out=bv_sb[:], in_=bvr[:])
            iqa = pp.tile([128, B * 32], BF16, tag="iqa")
            iqb = pp.tile([128, B * 32], BF16, tag="iqb")
            iv8 = pp.tile([128, B * B], BF16, tag="iv8")
            ibc0 = pp.tile([4, 128], BF16, tag="ibc0")
            ibc1 = pp.tile([4, 128], BF16, tag="ibc1")
            nc.sync.dma_start(out=iqa[:], in_=iqa_d[:])
            nc.sync.dma_start(out=iqb[:], in_=iqb_d[:])
            nc.sync.dma_start(out=iv8[:], in_=iv8_d[:])
            nc.sync.dma_start(out=ibc0[:], in_=ibc0_d[:])
            nc.sync.dma_start(out=ibc1[:], in_=ibc1_d[:])
            ones_rb = pp.tile([1, 128], BF16, tag="ones_rb")
            ones_rf = pp.tile([1, 128], F32R, tag="ones_rf")
            ones_col = pp.tile([128, 1], BF16, tag="ones_col")
            nc.sync.dma_start(out=ones_rb[:], in_=ones_rb_d[:])
            nc.sync.dma_start(out=ones_rf[:], in_=ones_rf_d[:])
            nc.sync.dma_start(out=ones_col[:], in_=ones_c_d[:])
            rrs_row = pp.tile([1, S], F32R, tag="rrs_row")
            nc.sync.dma_start(out=rrs_row[:], in_=rrs[:])

            # persistent SBUF tensors
            q_sb = [[pp.tile([128, S], BF16, tag=f"q{b}_{h}",
                             name=f"q{b}_{h}")
                     for h in range(2)] for b in range(B)]
            ss_all = pp.tile([32, S], F32, tag="ss_all")
            rq_all = pp.tile([32, S], F32, tag="rq_all")
            vmacc = [pp.tile([128, B * D], BF16, tag=f"vmacc{c}",
                             name=f"vmacc{c}")
                     for c in range(4)]
            attn_sb = [pp.tile([128, S], F32, tag=f"at{p}", name=f"at{p}")
                       for p in range(4)]
            rrs_bc = pp.tile([128, S], F32, tag="rrs_bc")

            with tc.tile_pool(name="psA", bufs=1, space="PSUM") as psA:
                # broadcast rrs row to all 128 partitions (K=1 matmul)
                pbc0 = psA.tile([128, S], F32, tag="pq", bufs=2)
                nc.tensor.matmul(pbc0[:], lhsT=ones_rf[:], rhs=rrs_row[:],
                                 start=True, stop=True)
                nc.scalar.activation(rrs_bc[:], pbc0[:], ACTF.Copy)

                # ============ phase A: k/v/kvs/vmean for all batches ========
                vs8_ps = psA.tile([B, HD], F32, tag="vs8")
                for b in range(B):
                    xs0 = wk.tile([128, S], BF16, tag="xs0", bufs=2)
                    xs1 = wk.tile([128, S], BF16, tag="xs1", bufs=2)
                    nc.sync.dma_start(out=xs0[:], in_=xs[b, 0])
                    nc.sync.dma_start(out=xs1[:], in_=xs[b, 1])

                    # one bank per kvs half: interleaved accumulation
                    # groups within one PSUM bank lose the first group's
                    # start contribution. col 256 = ks (ones column of vt)
                    kvs_e0 = psA.tile([128, HD + 1], F32, tag="kvse0")
                    kvs_e1 = psA.tile([128, HD + 1], F32, tag="kvse1")

                    for sb_i in range(4):
                        sl = slice(sb_i * 128, (sb_i + 1) * 128)
                        st = sb_i == 0
                        sp = sb_i == 3
                        pkv = psA.tile([128, 2 * HD], F32, tag="pkv",
                                       bufs=2)
                        pk = pkv[:, 0:HD]
                        pv = pkv[:, HD:2 * HD]
                        for (ps, wt, brow) in ((pk, wk_t, bk_sb),
                                               (pv, wv_t, bv_sb)):
                            nc.tensor.matmul(ps, lhsT=xs0[:, sl],
                                             rhs=wt[0][:], start=True,
                                             stop=False)
                            nc.tensor.matmul(ps, lhsT=xs1[:, sl],
                                             rhs=wt[1][:], start=False,
                                             stop=False)
                            nc.tensor.matmul(ps, lhsT=ones_rb[:],
                                             rhs=brow[:], start=False,
                                             stop=True)
                        # v^T in bf16 for kvs/vs matmuls; col 256 = ones
                        # so the kvs matmul also produces ks_sum
                        vt = wk.tile([128, HD + 2], BF16, tag="vt", bufs=2)
                        nc.scalar.activation(vt[:, 0:HD], pv, ACTF.Copy)
                        nc.vector.memset(vt[:, HD:HD + 1], 1.0)
                        # vmean (sum over heads; /4 folded into rrs) -> bf16
                        nc.vector.reduce_sum(
                            vmacc[sb_i][:, b * D:(b + 1) * D],
                            pv.rearrange("p (h d) -> p d h", h=H), axis=AX)
                        # kn = k / ||k|| (per head), bf16
                        sq = wk.tile([128, HD], F32, tag="sq", bufs=2)
                        nc.scalar.activation(sq[:], pk, ACTF.Square)
                        ssk = wk.tile([128, H], F32, tag="ssk", bufs=2)
                        nc.vector.reduce_sum(
                            ssk[:], sq[:].rearrange("p (h d) -> p h d", h=H),
                            axis=AX)
                        snk = wk.tile([128, H], F32, tag="snk", bufs=2)
                        nc.scalar.activation(snk[:], ssk[:], ACTF.Sqrt)
                        rk = wk.tile([128, H], F32, tag="rk", bufs=2)
                        nc.vector.reciprocal(rk[:], snk[:])
                        kt = wk.tile([128, HD], BF16, tag="kt", bufs=2)
                        for h in range(H):
                            dsl = slice(h * D, (h + 1) * D)
                            nc.vector.tensor_scalar_mul(
                                kt[:, dsl], pk[:, dsl], rk[:, h:h + 1])
                        # kvs+ks / vs partial accumulation
                        nc.tensor.matmul(kvs_e0[:], lhsT=kt[:, 0:128],
                                         rhs=vt[:, 0:HD + 1], start=st,
                                         stop=sp)
                        nc.tensor.matmul(kvs_e1[:], lhsT=kt[:, 128:HD],
                                         rhs=vt[:, 0:HD + 1], start=st,
                                         stop=sp)
                        # vs rows for all batches accumulate in one [B, HD]
                        nc.tensor.matmul(vs8_ps[:],
                                         lhsT=iv8[:, b * B:(b + 1) * B],
                                         rhs=vt[:, 0:HD],
                                         start=(b == 0 and st),
                                         stop=(b == B - 1 and sp))

                    # evac kvs diag blocks + ks column -> AR1 payload
                    pk0 = wk.tile([128, 65], F32, tag="arpk0", bufs=2)
                    pk1 = wk.tile([128, 65], F32, tag="arpk1", bufs=2)
                    nc.scalar.activation(pk0[0:64, 0:64], kvs_e0[0:64, 0:64],
                                         ACTF.Copy)
                    nc.scalar.activation(pk0[64:128, 0:64],
                                         kvs_e0[64:128, 64:128], ACTF.Copy)
                    nc.scalar.activation(pk0[:, 64:65], kvs_e0[:, HD:HD + 1],
                                         ACTF.Copy)
                    nc.scalar.activation(pk1[0:64, 0:64],
                                         kvs_e1[0:64, 128:192], ACTF.Copy)
                    nc.scalar.activation(pk1[64:128, 0:64],
                                         kvs_e1[64:128, 192:256], ACTF.Copy)
                    nc.scalar.activation(pk1[:, 64:65], kvs_e1[:, HD:HD + 1],
                                         ACTF.Copy)
                    nc.sync.dma_start(out=ar1_in[0, :, b, :], in_=pk0[:])
                    nc.sync.dma_start(out=ar1_in[1, :, b, :], in_=pk1[:])

                # vs rows + vmean chunks to DRAM
                vs8_sb = wk.tile([B, HD], F32, tag="vs8sb", bufs=1)
                nc.scalar.activation(vs8_sb[:], vs8_ps[:], ACTF.Copy)
                nc.sync.dma_start(out=ar2_in[:], in_=vs8_sb[:])
                for c in range(4):
                    nc.sync.dma_start(
                        out=vm_loc[c * 128:(c + 1) * 128, :, :],
                        in_=vmacc[c][:])

                # ================= collectives (overlap with q work) ========
                nc.gpsimd.collective_compute(
                    "AllReduce", ALU.add, ins=[ar1_in.opt()],
                    outs=[ar1_out.opt()], replica_groups=RG)
                nc.gpsimd.collective_compute(
                    "AllReduce", ALU.add, ins=[ar2_in.opt()],
                    outs=[ar2_out.opt()], replica_groups=RG)
                nc.gpsimd.collective_compute(
                    "AllGather", ALU.bypass, ins=[vm_loc.opt()],
                    outs=[vm_all.opt()], replica_groups=RG)

                # ============ phase B: q projections (under collectives) ====
                ss32_ps = psA.tile([32, S], F32, tag="ss32")
                for b in range(B):
                    xq0 = wk.tile([128, S], BF16, tag="xs0", bufs=2)
                    xq1 = wk.tile([128, S], BF16, tag="xs1", bufs=2)
                    nc.sync.dma_start(out=xq0[:], in_=xq[b, 0])
                    nc.sync.dma_start(out=xq1[:], in_=xq[b, 1])
                    for h in range(2):
                        hsl = slice(h * 128, (h + 1) * 128)
                        pq = psA.tile([128, S], F32, tag="pq", bufs=2)
                        nc.tensor.matmul(pq[:], lhsT=wq_t[0][:, hsl],
                                         rhs=xq0[:], start=True, stop=False)
                        nc.tensor.matmul(pq[:], lhsT=wq_t[1][:, hsl],
                                         rhs=xq1[:], start=False, stop=True)
                        nc.scalar.add(q_sb[b][h][:], pq[:],
                                      bq_sb[:, h:h + 1])
                        qsq = wk.tile([128, S], BF16, tag="qsq", bufs=2)
                        nc.scalar.activation(qsq[:], q_sb[b][h][:],
                                             ACTF.Square)
                        # |q|^2 rows land at [4b..4b+3] via batch indicator
                        nc.tensor.matmul(
                            ss32_ps[:],
                            lhsT=(iqa if h == 0 else iqb)[:,
                                                          b * 32:(b + 1) * 32],
                            rhs=qsq[:], start=(b == 0 and h == 0),
                            stop=(b == B - 1 and h == 1))
                # rq = 1/|q| for all batches in one op
                nc.scalar.activation(ss_all[:], ss32_ps[:], ACTF.Sqrt)
                nc.vector.reciprocal_approx_fast(rq_all[:], ss_all[:])

            # ============ phase C: attention epilogue ======================
            with tc.tile_pool(name="psB", bufs=2, space="PSUM") as psB:
                kpk_bf = []
                for h2 in range(2):
                    kf = wk.tile([128, B * 65], F32, tag=f"kf{h2}", bufs=1,
                                 name=f"kf{h2}")
                    nc.sync.dma_start(out=kf[:], in_=ar1_out[h2])
                    kb = pp.tile([128, B * 65], BF16, tag=f"kpkb{h2}",
                                 name=f"kpkb{h2}")
                    nc.scalar.activation(kb[:], kf[:], ACTF.Copy)
                    kpk_bf.append(kb)
                vs8f = wk.tile([B, HD], F32, tag="vs8f", bufs=1)
                nc.sync.dma_start(out=vs8f[:], in_=ar2_out[:])
                vs8b = wk.tile([B, HD], BF16, tag="vs8b", bufs=1)
                nc.scalar.activation(vs8b[:], vs8f[:], ACTF.Copy)
                # vs rows repacked to [4, B*64] (lhsT needs base partition 0)
                vs4 = wk.tile([4, B * D], BF16, tag="vs4", bufs=1)
                for b in range(B):
                    nc.sync.dma_start(
                        out=vs4[:, b * D:(b + 1) * D],
                        in_=vs8b[b:b + 1, :])

                # masked ks tiles: block b col (4b+2*h2+hh) = head hh of
                # half h2 for batch b (zeros elsewhere)
                ksm = []
                for h2 in range(2):
                    km = wk.tile([128, 32 * B], BF16, tag=f"ksm{h2}", bufs=1,
                                 name=f"ksm{h2}")
                    nc.vector.memset(km[:], 0.0)
                    for b in range(B):
                        col = b * 65 + 64
                        for hh in range(2):
                            rs = slice(hh * 64, hh * 64 + 64)
                            dst = b * 32 + 4 * b + 2 * h2 + hh
                            nc.scalar.activation(
                                km[rs, dst:dst + 1],
                                kpk_bf[h2][rs, col:col + 1], ACTF.Copy)
                    ksm.append(km)
                # denominators: pden32[4b+j] = q_j . ks_j (accumulated)
                pden32 = psB.tile([32, S], F32, tag="pden", bufs=1)
                for b in range(B):
                    nc.tensor.matmul(pden32[:],
                                     lhsT=ksm[0][:, b * 32:(b + 1) * 32],
                                     rhs=q_sb[b][0][:], start=(b == 0),
                                     stop=False)
                    nc.tensor.matmul(pden32[:],
                                     lhsT=ksm[1][:, b * 32:(b + 1) * 32],
                                     rhs=q_sb[b][1][:], start=False,
                                     stop=(b == B - 1))
                t0 = wk.tile([32, S], F32, tag="t0", bufs=1)
                nc.vector.tensor_mul(t0[:], pden32[:], rq_all[:])
                t1 = wk.tile([32, S], F32, tag="t1", bufs=1)
                nc.vector.tensor_scalar(t1[:], t0[:], 4.0, float(4 * N),
                                        op0=ALU.mult, op1=ALU.add)
                rp_f = wk.tile([32, S], F32, tag="rpf", bufs=1)
                nc.vector.reciprocal_approx_fast(rp_f[:], t1[:])
                cc_f = wk.tile([32, S], F32, tag="ccf", bufs=1)
                nc.vector.tensor_mul(cc_f[:], rp_f[:], rq_all[:])
                rp_b32 = wk.tile([32, S], BF16, tag="rpb32", bufs=1)
                cc_b32 = wk.tile([32, S], BF16, tag="ccb32", bufs=1)
                nc.scalar.activation(rp_b32[:], rp_f[:], ACTF.Copy)
                nc.scalar.activation(cc_b32[:], cc_f[:], ACTF.Copy)
                # repack to [4, B*S] via SBUF->SBUF DMA (base partition 0)
                rp4 = wk.tile([4, B * S], BF16, tag="rp4", bufs=1)
                cc4 = wk.tile([4, B * S], BF16, tag="cc4", bufs=1)
                for b in range(B):
                    cs = slice(b * S, (b + 1) * S)
                    nc.sync.dma_start(out=rp4[:, cs],
                                      in_=rp_b32[4 * b:4 * b + 4, :])
                    nc.sync.dma_start(out=cc4[:, cs],
                                      in_=cc_b32[4 * b:4 * b + 4, :])

                for b in range(B):
                    pat = psB.tile([D, S], F32, tag="pat", bufs=1)
                    for h2 in range(2):
                        pbc = psB.tile([128, S], F32, tag="pbc")
                        nc.tensor.matmul(
                            pbc[:],
                            lhsT=(ibc0[:] if h2 == 0 else ibc1[:]),
                            rhs=cc4[:, b * S:(b + 1) * S],
                            start=True, stop=True)
                        qs = wk.tile([128, S], BF16, tag="qs", bufs=2)
                        nc.vector.tensor_mul(qs[:], q_sb[b][h2][:], pbc[:])
                        nc.tensor.matmul(
                            pat[:],
                            lhsT=kpk_bf[h2][:, b * 65:b * 65 + 64],
                            rhs=qs[:], start=(h2 == 0), stop=False)
                    nc.tensor.matmul(pat[:], lhsT=vs4[:, b * D:(b + 1) * D],
                                     rhs=rp4[:, b * S:(b + 1) * S],
                                     start=False, stop=True)
                    nc.scalar.activation(
                        attn_sb[b // 2][(b % 2) * D:(b % 2 + 1) * D, :],
                        pat[:], ACTF.Copy)

                if DBG:
                    nc.sync.dma_start(out=dbg1[:], in_=ar1_out[:])
                # ================= phase D: GCN ============================
                with tc.tile_pool(name="psC", bufs=1, space="PSUM") as psC:
                    pg = [psC.tile([128, S], F32, tag=f"g{p}", name=f"g{p}")
                          for p in range(4)]
                    for mc in range(32):
                        adj_t = wk.tile([128, S], BF16, tag="adj", bufs=2)
                        nc.sync.dma_start(out=adj_t[:], in_=adjt[mc])
                        vm_t = wk.tile([128, B * D], BF16, tag="vml", bufs=2)
                        lc = mc % 4
                        nc.sync.dma_start(
                            out=vm_t[:],
                            in_=vm_all[mc // 4, lc * 128:(lc + 1) * 128,
                                       :, :])
                        for p in range(4):
                            nc.tensor.matmul(
                                pg[p][:],
                                lhsT=vm_t[:, p * 128:(p + 1) * 128],
                                rhs=adj_t[:], start=(mc == 0),
                                stop=(mc == 31))
                    for p in range(4):
                        gt = wk.tile([128, S], F32, tag="gt", bufs=2)
                        nc.vector.tensor_mul(gt[:], pg[p][:], rrs_bc[:])
                        ot = wk.tile([128, S], F32, tag="ot", bufs=2)
                        nc.vector.tensor_add(ot[:], gt[:], attn_sb[p][:])
                        nc.sync.dma_start(out=out[2 * p], in_=ot[0:D, :])
                        nc.sync.dma_start(out=out[2 * p + 1], in_=ot[D:128, :])
    nc.compile()
    return nc


def _prep_inputs(query_input, source_input, adj, Wq_w, Wq_b, Wk_w, Wk_b,
                 Wv_w, Wv_b):
    bf = ml_dtypes.bfloat16
    xq_np = np.asarray(query_input, dtype=np.float32)
    xs_np = np.asarray(source_input, dtype=np.float32)
    adj_np = np.asarray(adj, dtype=np.float32)

    adjT = np.ascontiguousarray(adj_np.T)
    np.fill_diagonal(adjT, adjT.diagonal() + 1.0)
    adjT_bf = adjT.astype(bf)
    rrs_full = (0.25 / (adj_np.sum(axis=1) + 1.0)).astype(np.float32)

    wqt = np.ascontiguousarray(np.asarray(Wq_w, np.float32).T).reshape(
        2, 128, HD).astype(bf)
    wkt = np.ascontiguousarray(np.asarray(Wk_w, np.float32).T).reshape(
        2, 128, HD).astype(bf)
    wvt = np.ascontiguousarray(np.asarray(Wv_w, np.float32).T).reshape(
        2, 128, HD).astype(bf)
    bqc = np.ascontiguousarray(
        np.asarray(Wq_b, np.float32).reshape(2, 128).T)
    bk = np.asarray(Wk_b, np.float32).reshape(1, HD).astype(bf)
    bv = np.asarray(Wv_b, np.float32).reshape(1, HD).astype(bf)

    # batch-indicator tiles for [32,S]/[B,HD] accumulating matmuls
    iqa = np.zeros((128, B * 32), np.float32)
    iqb = np.zeros((128, B * 32), np.float32)
    iv8 = np.zeros((128, B * B), np.float32)
    for b in range(B):
        for p in range(128):
            iqa[p, b * 32 + 4 * b + p // 64] = 1.0
            iqb[p, b * 32 + 4 * b + 2 + p // 64] = 1.0
            iv8[p, b * B + b] = 1.0
    ibc0 = np.zeros((4, 128), np.float32)
    ibc1 = np.zeros((4, 128), np.float32)
    for p in range(128):
        ibc0[p // 64, p] = 1.0
        ibc1[2 + p // 64, p] = 1.0

    xq_bf = xq_np.astype(bf)
    xs_bf = xs_np.astype(bf)
    in_maps = []
    for i in range(NCORES):
        sl = slice(i * S, (i + 1) * S)
        in_maps.append({
            "xq": np.ascontiguousarray(xq_bf[:, :, sl]).reshape(B, 2, 128, S),
            "xs": np.ascontiguousarray(xs_bf[:, :, sl]).reshape(B, 2, 128, S),
            "adjt": np.ascontiguousarray(adjT_bf[:, sl]).reshape(32, 128, S),
            "rrs": np.ascontiguousarray(rrs_full[sl]).reshape(1, S),
            "wqt": wqt, "wkt": wkt, "wvt": wvt,
            "bqc": bqc, "bkr": bk, "bvr": bv,
            "iqa_in": iqa.astype(bf), "iqb_in": iqb.astype(bf),
            "iv8_in": iv8.astype(bf),
            "ibc0_in": ibc0.astype(bf), "ibc1_in": ibc1.astype(bf),
            "ones_rb": np.ones((1, 128), bf),
            "ones_rf": np.ones((1, 128), np.float32),
            "ones_c": np.ones((128, 1), bf),
        })
    return in_maps


def kernel(**inputs):
    if "nc" not in _CACHE:
        _CACHE["nc"] = _build()
    nc = _CACHE["nc"]
    in_maps = _prep_inputs(**inputs)
    res = run_bass_kernel_spmd(nc, in_maps, list(range(NCORES)))
    full = np.empty((B, D, N), np.float32)
    for i in range(NCORES):
        full[:, :, i * S:(i + 1) * S] = res.results[i]["out"]
    return full


# revision 11
# speedup vs baseline: 1.7385x; 1.1475x over previous
"""DIFFormerConv (simple linear attention + dense GCN) on 8 trn2 NeuronCores.

Sharding: nodes N=4096 split 8 ways (S=512 per core). Each core computes
q/k/v for its node shard, partial kvs/ks_sum/vsum (AllReduce), vmean
(AllGather, bf16), the attention output rows for its shard, and the GCN
rows for its shard (adj^T column shard, bf16 matmul).

All matmuls run in bf16 (full PE rate). Phase order maximizes
collective overlap: k/v/kvs/vmean for all batches -> AllReduce +
AllGather issued -> q projections run under the collectives -> attention
epilogue (batched denominators) -> GCN.

Engines can only address SBUF/PSUM at partition offsets 0/32/64, so all
per-batch [4,*] rows are produced via indicator matmuls accumulating
into batched [32,S]/[B,HD] PSUM tiles, and repacked to [4, B*S] layout
with small SBUF->SBUF DMAs where matmul operands need base partition 0.

Layouts (no PE transposes needed):
  q:   [hd, s]  (heads*dim on partitions)  -- lhsT = W^T chunks, bias via
                                              per-partition activation add
  k,v: [s, hd]  (transposed projection)    -- lhsT = x chunks, bias via
                                              K=1 ones matmul
  kvs AllReduce payload: [2, 128, B, 65] f32 (64 diag cols + ks column)
  gcn: [(b,d), n] directly                 -- lhsT = vmean[m,(b,d)],
                                              rhs = adjT[m,n], bf16
Host prep: adjT = adj.T + I (bf16), rrs = 0.25/(rowsum+1), W^T in bf16.
"""

import sys

sys.path.insert(0, "/opt/trn_rl_repo")

import numpy as np
import ml_dtypes

from concourse import bass, bacc, tile, mybir
from concourse.bass_utils import run_bass_kernel_spmd

B, C, N, H, D = 8, 256, 4096, 4, 64
NCORES = 8
S = N // NCORES          # 512 nodes per core
HD = H * D               # 256
F32 = mybir.dt.float32
F32R = mybir.dt.float32r
BF16 = mybir.dt.bfloat16
AX = mybir.AxisListType.X
ALU = mybir.AluOpType
ACTF = mybir.ActivationFunctionType
RG = [list(range(NCORES))]

_CACHE = {}


def _build():
    nc = bacc.Bacc("TRN2", target_bir_lowering=False, debug=False,
                   num_devices=NCORES)

    xq = nc.dram_tensor("xq", [B, 2, 128, S], BF16, kind="ExternalInput")
    xs = nc.dram_tensor("xs", [B, 2, 128, S], BF16, kind="ExternalInput")
    adjt = nc.dram_tensor("adjt", [32, 128, S], BF16, kind="ExternalInput")
    rrs = nc.dram_tensor("rrs", [1, S], F32R, kind="ExternalInput")
    wqt = nc.dram_tensor("wqt", [2, 128, HD], BF16, kind="ExternalInput")
    wkv = nc.dram_tensor("wkv", [2, 128, 2 * HD], BF16, kind="ExternalInput")
    bqc = nc.dram_tensor("bqc", [128, 2], F32, kind="ExternalInput")
    bkv = nc.dram_tensor("bkv", [1, 2 * HD], BF16, kind="ExternalInput")
    out = nc.dram_tensor("out", [B, D, S], F32, kind="ExternalOutput")
    import os
    DBG = bool(os.environ.get("KDBG"))
    if DBG:
        dbg1 = nc.dram_tensor("dbg1", [2, 128, B, 65], BF16,
                              kind="ExternalOutput")

    # indicator tensors (see _prep_inputs)
    iqa_d = nc.dram_tensor("iqa_in", [128, B * 32], BF16, kind="ExternalInput")
    iqb_d = nc.dram_tensor("iqb_in", [128, B * 32], BF16, kind="ExternalInput")
    iv8_d = nc.dram_tensor("iv8_in", [128, B * B], BF16, kind="ExternalInput")
    ibc0_d = nc.dram_tensor("ibc0_in", [4, 128], BF16, kind="ExternalInput")
    ibc1_d = nc.dram_tensor("ibc1_in", [4, 128], BF16, kind="ExternalInput")
    ones_rb_d = nc.dram_tensor("ones_rb", [1, 128], BF16, kind="ExternalInput")
    ones_rf_d = nc.dram_tensor("ones_rf", [1, 128], F32R, kind="ExternalInput")
    ones_c_d = nc.dram_tensor("ones_c", [128, 1], BF16, kind="ExternalInput")

    with nc.allow_low_precision(reason="bf16 matmul pipeline intentional"), \
            tile.TileContext(nc) as tc:
        with (
            tc.tile_pool(name="pers", bufs=1) as pp,
            tc.tile_pool(name="work", bufs=3) as wk,
            tc.tile_pool(name="dram", bufs=1, space="DRAM") as dp,
        ):
            # DRAM internal buffers for collectives
            vm_loc = dp.tile([S, B, D], BF16, tag="vm_loc", name="vm_loc")
            vm_all = dp.tile([NCORES, S, B, D], BF16, tag="vm_all",
                             name="vm_all", addr_space="Shared")
            ar1_in = dp.tile([2, 128, B, 65], BF16, tag="ar1_in",
                             name="ar1_in")
            ar1_out = dp.tile([2, 128, B, 65], BF16, tag="ar1_out",
                              name="ar1_out", addr_space="Shared")
            ar2_in = dp.tile([B, HD], BF16, tag="ar2_in", name="ar2_in")
            ar2_out = dp.tile([B, HD], BF16, tag="ar2_out", name="ar2_out",
                              addr_space="Shared")

            # ---- constants ----
            wq_t = [pp.tile([128, HD], BF16, tag=f"wq{c}", name=f"wq{c}")
                    for c in range(2)]
            wkv_t = [pp.tile([128, 2 * HD], BF16, tag=f"wkv{c}",
                             name=f"wkv{c}") for c in range(2)]
            for c in range(2):
                nc.sync.dma_start(out=wq_t[c][:], in_=wqt[c])
                nc.sync.dma_start(out=wkv_t[c][:], in_=wkv[c])
            bq_sb = pp.tile([128, 2], F32, tag="bqsb")
            bkv_sb = pp.tile([1, 2 * HD], BF16, tag="bkvsb")
            nc.sync.dma_start(out=bq_sb[:], in_=bqc[:])
            nc.sync.dma_start(out=bkv_sb[:], in_=bkv[:])
            iqa = pp.tile([128, B * 32], BF16, tag="iqa")
            iqb = pp.tile([128, B * 32], BF16, tag="iqb")
            iv8 = pp.tile([128, B * B], BF16, tag="iv8")
            ibc0 = pp.tile([4, 128], BF16, tag="ibc0")
            ibc1 = pp.tile([4, 128], BF16, tag="ibc1")
            nc.sync.dma_start(out=iqa[:], in_=iqa_d[:])
            nc.sync.dma_start(out=iqb[:], in_=iqb_d[:])
            nc.sync.dma_start(out=iv8[:], in_=iv8_d[:])
            nc.sync.dma_start(out=ibc0[:], in_=ibc0_d[:])
            nc.sync.dma_start(out=ibc1[:], in_=ibc1_d[:])
            ones_rb = pp.tile([1, 128], BF16, tag="ones_rb")
            ones_rf = pp.tile([1, 128], F32R, tag="ones_rf")
            ones_col = pp.tile([128, 1], BF16, tag="ones_col")
            nc.sync.dma_start(out=ones_rb[:], in_=ones_rb_d[:])
            nc.sync.dma_start(out=ones_rf[:], in_=ones_rf_d[:])
            nc.sync.dma_start(out=ones_col[:], in_=ones_c_d[:])
            rrs_row = pp.tile([1, S], F32R, tag="rrs_row")
            nc.sync.dma_start(out=rrs_row[:], in_=rrs[:])

            # persistent SBUF tensors
            q_sb = [[pp.tile([128, S], BF16, tag=f"q{b}_{h}",
                             name=f"q{b}_{h}")
                     for h in range(2)] for b in range(B)]
            ss_all = pp.tile([32, S], F32, tag="ss_all")
            rq_all = pp.tile([32, S], F32, tag="rq_all")
            vmacc = [pp.tile([128, B * D], BF16, tag=f"vmacc{c}",
                             name=f"vmacc{c}")
                     for c in range(4)]
            attn_sb = [pp.tile([128, S], F32, tag=f"at{p}", name=f"at{p}")
                       for p in range(4)]
            rrs_bc = pp.tile([128, S], F32, tag="rrs_bc")

            with tc.tile_pool(name="psA", bufs=1, space="PSUM") as psA:
                # broadcast rrs row to all 128 partitions (K=1 matmul)
                pbc0 = psA.tile([128, S], F32, tag="pq", bufs=2)
                nc.tensor.matmul(pbc0[:], lhsT=ones_rf[:], rhs=rrs_row[:],
                                 start=True, stop=True)
                nc.scalar.activation(rrs_bc[:], pbc0[:], ACTF.Copy)

                # ============ phase A: k/v/kvs/vmean for all batches ========
                # merged k|v projection ([128,512] matmuls), then batched
                # normalization per batch so the PE never waits on the
                # scalar/vector normalize chain chunk-by-chunk
                vs8_ps = psA.tile([B, HD], F32, tag="vs8")
                VW = HD + 1  # vt chunk stride (col 256 of each chunk = ones)
                for b in range(B):
                    xs0 = wk.tile([128, S], BF16, tag="xs0", bufs=2)
                    xs1 = wk.tile([128, S], BF16, tag="xs1", bufs=2)
                    nc.sync.dma_start(out=xs0[:], in_=xs[b, 0])
                    nc.sync.dma_start(out=xs1[:], in_=xs[b, 1])

                    k_all = wk.tile([128, 4 * HD], BF16, tag="k_all", bufs=2)
                    vt_all = wk.tile([128, 4 * VW], BF16, tag="vt_all",
                                     bufs=2)
                    for c4 in range(4):
                        sl = slice(c4 * 128, (c4 + 1) * 128)
                        pkv = psA.tile([128, 2 * HD], F32, tag="pkv",
                                       bufs=2)
                        nc.tensor.matmul(pkv[:], lhsT=xs0[:, sl],
                                         rhs=wkv_t[0][:], start=True,
                                         stop=False)
                        nc.tensor.matmul(pkv[:], lhsT=xs1[:, sl],
                                         rhs=wkv_t[1][:], start=False,
                                         stop=False)
                        nc.tensor.matmul(pkv[:], lhsT=ones_rb[:],
                                         rhs=bkv_sb[:], start=False,
                                         stop=True)
                        nc.scalar.activation(
                            k_all[:, c4 * HD:(c4 + 1) * HD], pkv[:, 0:HD],
                            ACTF.Copy)
                        nc.vector.tensor_scalar_mul(
                            vt_all[:, c4 * VW:c4 * VW + HD],
                            pkv[:, HD:2 * HD], 1.0)
                        nc.vector.memset(
                            vt_all[:, c4 * VW + HD:c4 * VW + HD + 1], 1.0)
                        nc.vector.reduce_sum(
                            vmacc[c4][:, b * D:(b + 1) * D],
                            pkv[:, HD:2 * HD].rearrange("p (h d) -> p d h",
                                                        h=H), axis=AX)
                    # batched normalization for all 4 chunks x 4 heads
                    sq = wk.tile([128, 4 * HD], F32, tag="sq", bufs=2)
                    nc.scalar.activation(sq[:], k_all[:], ACTF.Square)
                    ssk = wk.tile([128, 16], F32, tag="ssk", bufs=2)
                    nc.vector.reduce_sum(
                        ssk[:], sq[:].rearrange("p (g d) -> p g d", g=16),
                        axis=AX)
                    snk = wk.tile([128, 16], F32, tag="snk", bufs=2)
                    nc.scalar.activation(snk[:], ssk[:], ACTF.Sqrt)
                    rk = wk.tile([128, 16], F32, tag="rk", bufs=2)
                    nc.vector.reciprocal(rk[:], snk[:])
                    kt_all = wk.tile([128, 4 * HD], BF16, tag="kt_all",
                                     bufs=2)
                    for g in range(16):
                        gs = slice(g * D, (g + 1) * D)
                        nc.vector.tensor_scalar_mul(
                            kt_all[:, gs], k_all[:, gs], rk[:, g:g + 1])

                    # kvs+ks / vs accumulation (one PSUM bank per half --
                    # interleaved groups within one bank lose the first
                    # group's start contribution)
                    kvs_e0 = psA.tile([128, HD + 1], F32, tag="kvse0")
                    kvs_e1 = psA.tile([128, HD + 1], F32, tag="kvse1")
                    for c4 in range(4):
                        st = c4 == 0
                        sp = c4 == 3
                        vchunk = vt_all[:, c4 * VW:c4 * VW + HD + 1]
                        nc.tensor.matmul(kvs_e0[:],
                                         lhsT=kt_all[:, c4 * HD:c4 * HD + 128],
                                         rhs=vchunk, start=st, stop=sp)
                        nc.tensor.matmul(kvs_e1[:],
                                         lhsT=kt_all[:,
                                                     c4 * HD + 128:c4 * HD + HD],
                                         rhs=vchunk, start=st, stop=sp)
                        nc.tensor.matmul(vs8_ps[:],
                                         lhsT=iv8[:, b * B:(b + 1) * B],
                                         rhs=vt_all[:, c4 * VW:c4 * VW + HD],
                                         start=(b == 0 and st),
                                         stop=(b == B - 1 and sp))

                    # evac kvs diag blocks + ks column -> AR1 payload (bf16)
                    pk0 = wk.tile([128, 65], BF16, tag="arpk0", bufs=2)
                    pk1 = wk.tile([128, 65], BF16, tag="arpk1", bufs=2)
                    nc.scalar.activation(pk0[0:64, 0:64], kvs_e0[0:64, 0:64],
                                         ACTF.Copy)
                    nc.scalar.activation(pk0[64:128, 0:64],
                                         kvs_e0[64:128, 64:128], ACTF.Copy)
                    nc.scalar.activation(pk0[:, 64:65], kvs_e0[:, HD:HD + 1],
                                         ACTF.Copy)
                    nc.scalar.activation(pk1[0:64, 0:64],
                                         kvs_e1[0:64, 128:192], ACTF.Copy)
                    nc.scalar.activation(pk1[64:128, 0:64],
                                         kvs_e1[64:128, 192:256], ACTF.Copy)
                    nc.scalar.activation(pk1[:, 64:65], kvs_e1[:, HD:HD + 1],
                                         ACTF.Copy)
                    nc.sync.dma_start(out=ar1_in[0, :, b, :], in_=pk0[:])
                    nc.sync.dma_start(out=ar1_in[1, :, b, :], in_=pk1[:])

                # vs rows + vmean chunks to DRAM
                vs8_sb = wk.tile([B, HD], BF16, tag="vs8sb", bufs=1)
                nc.scalar.activation(vs8_sb[:], vs8_ps[:], ACTF.Copy)
                nc.sync.dma_start(out=ar2_in[:], in_=vs8_sb[:])
                for c in range(4):
                    nc.sync.dma_start(
                        out=vm_loc[c * 128:(c + 1) * 128, :, :],
                        in_=vmacc[c][:])

                # ================= collectives (overlap with q work) ========
                nc.gpsimd.collective_compute(
                    "AllReduce", ALU.add, ins=[ar1_in.opt()],
                    outs=[ar1_out.opt()], replica_groups=RG)
                nc.gpsimd.collective_compute(
                    "AllReduce", ALU.add, ins=[ar2_in.opt()],
                    outs=[ar2_out.opt()], replica_groups=RG)
                nc.gpsimd.collective_compute(
                    "AllGather", ALU.bypass, ins=[vm_loc.opt()],
                    outs=[vm_all.opt()], replica_groups=RG)

                # ============ phase B: q projections (under collectives) ====
                ss32_ps = psA.tile([32, S], F32, tag="ss32")
                for b in range(B):
                    xq0 = wk.tile([128, S], BF16, tag="xs0", bufs=2)
                    xq1 = wk.tile([128, S], BF16, tag="xs1", bufs=2)
                    nc.sync.dma_start(out=xq0[:], in_=xq[b, 0])
                    nc.sync.dma_start(out=xq1[:], in_=xq[b, 1])
                    for h in range(2):
                        hsl = slice(h * 128, (h + 1) * 128)
                        pq = psA.tile([128, S], F32, tag="pq", bufs=2)
                        nc.tensor.matmul(pq[:], lhsT=wq_t[0][:, hsl],
                                         rhs=xq0[:], start=True, stop=False)
                        nc.tensor.matmul(pq[:], lhsT=wq_t[1][:, hsl],
                                         rhs=xq1[:], start=False, stop=True)
                        nc.vector.tensor_scalar(
                            q_sb[b][h][:], pq[:], bq_sb[:, h:h + 1], None,
                            op0=ALU.add)
                        qsq = wk.tile([128, S], BF16, tag="qsq", bufs=2)
                        nc.scalar.activation(qsq[:], pq[:], ACTF.Square,
                                             bias=bq_sb[:, h:h + 1])
                        # |q|^2 rows land at [4b..4b+3] via batch indicator
                        nc.tensor.matmul(
                            ss32_ps[:],
                            lhsT=(iqa if h == 0 else iqb)[:,
                                                          b * 32:(b + 1) * 32],
                            rhs=qsq[:], start=(b == 0 and h == 0),
                            stop=(b == B - 1 and h == 1))
                # rq = 1/|q| for all batches in one op
                nc.scalar.activation(ss_all[:], ss32_ps[:], ACTF.Sqrt)
                nc.vector.reciprocal_approx_fast(rq_all[:], ss_all[:])

            # ============ phase C: attention epilogue ======================
            with tc.tile_pool(name="psB", bufs=2, space="PSUM") as psB:
                kpk_bf = []
                for h2 in range(2):
                    kf = wk.tile([128, B * 65], BF16, tag=f"kf{h2}", bufs=1,
                                 name=f"kf{h2}")
                    nc.sync.dma_start(out=kf[:], in_=ar1_out[h2])
                    kpk_bf.append(kf)
                vs8b = wk.tile([B, HD], BF16, tag="vs8b", bufs=1)
                nc.sync.dma_start(out=vs8b[:], in_=ar2_out[:])
                # vs rows repacked to [4, B*64] (lhsT needs base partition 0)
                vs4 = wk.tile([4, B * D], BF16, tag="vs4", bufs=1)
                for b in range(B):
                    nc.sync.dma_start(
                        out=vs4[:, b * D:(b + 1) * D],
                        in_=vs8b[b:b + 1, :])

                # masked ks tiles: block b col (4b+2*h2+hh) = head hh of
                # half h2 for batch b (zeros elsewhere)
                ksm = []
                for h2 in range(2):
                    km = wk.tile([128, 32 * B], BF16, tag=f"ksm{h2}", bufs=1,
                                 name=f"ksm{h2}")
                    nc.vector.memset(km[:], 0.0)
                    for b in range(B):
                        col = b * 65 + 64
                        for hh in range(2):
                            rs = slice(hh * 64, hh * 64 + 64)
                            dst = b * 32 + 4 * b + 2 * h2 + hh
                            nc.scalar.activation(
                                km[rs, dst:dst + 1],
                                kpk_bf[h2][rs, col:col + 1], ACTF.Copy)
                    ksm.append(km)
                # denominators: pden32[4b+j] = q_j . ks_j (accumulated)
                pden32 = psB.tile([32, S], F32, tag="pden", bufs=1)
                for b in range(B):
                    nc.tensor.matmul(pden32[:],
                                     lhsT=ksm[0][:, b * 32:(b + 1) * 32],
                                     rhs=q_sb[b][0][:], start=(b == 0),
                                     stop=False)
                    nc.tensor.matmul(pden32[:],
                                     lhsT=ksm[1][:, b * 32:(b + 1) * 32],
                                     rhs=q_sb[b][1][:], start=False,
                                     stop=(b == B - 1))
                t0 = wk.tile([32, S], F32, tag="t0", bufs=1)
                nc.vector.tensor_mul(t0[:], pden32[:], rq_all[:])
                t1 = wk.tile([32, S], F32, tag="t1", bufs=1)
                nc.vector.tensor_scalar(t1[:], t0[:], 4.0, float(4 * N),
                                        op0=ALU.mult, op1=ALU.add)
                rp_f = wk.tile([32, S], F32, tag="rpf", bufs=1)
                nc.vector.reciprocal_approx_fast(rp_f[:], t1[:])
                cc_f = wk.tile([32, S], F32, tag="ccf", bufs=1)
                nc.vector.tensor_mul(cc_f[:], rp_f[:], rq_all[:])
                rp_b32 = wk.tile([32, S], BF16, tag="rpb32", bufs=1)
                cc_b32 = wk.tile([32, S], BF16, tag="ccb32", bufs=1)
                nc.scalar.activation(rp_b32[:], rp_f[:], ACTF.Copy)
                nc.scalar.activation(cc_b32[:], cc_f[:], ACTF.Copy)
                # repack to [4, B*S] via SBUF->SBUF DMA (base partition 0)
                rp4 = wk.tile([4, B * S], BF16, tag="rp4", bufs=1)
                cc4 = wk.tile([4, B * S], BF16, tag="cc4", bufs=1)
                for b in range(B):
                    cs = slice(b * S, (b + 1) * S)
                    nc.sync.dma_start(out=rp4[:, cs],
                                      in_=rp_b32[4 * b:4 * b + 4, :])
                    nc.sync.dma_start(out=cc4[:, cs],
                                      in_=cc_b32[4 * b:4 * b + 4, :])

                for b in range(B):
                    pat = psB.tile([D, S], F32, tag="pat", bufs=1)
                    for h2 in range(2):
                        pbc = psB.tile([128, S], F32, tag="pbc")
                        nc.tensor.matmul(
                            pbc[:],
                            lhsT=(ibc0[:] if h2 == 0 else ibc1[:]),
                            rhs=cc4[:, b * S:(b + 1) * S],
                            start=True, stop=True)
                        qs = wk.tile([128, S], BF16, tag="qs", bufs=2)
                        nc.vector.tensor_mul(qs[:], q_sb[b][h2][:], pbc[:])
                        nc.tensor.matmul(
                            pat[:],
                            lhsT=kpk_bf[h2][:, b * 65:b * 65 + 64],
                            rhs=qs[:], start=(h2 == 0), stop=False)
                    nc.tensor.matmul(pat[:], lhsT=vs4[:, b * D:(b + 1) * D],
                                     rhs=rp4[:, b * S:(b + 1) * S],
                                     start=False, stop=True)
                    nc.scalar.activation(
                        attn_sb[b // 2][(b % 2) * D:(b % 2 + 1) * D, :],
                        pat[:], ACTF.Copy)

                if DBG:
                    nc.sync.dma_start(out=dbg1[:], in_=ar1_out[:])
                # ================= phase D: GCN ============================
                with tc.tile_pool(name="psC", bufs=1, space="PSUM") as psC:
                    pg = [psC.tile([128, S], F32, tag=f"g{p}", name=f"g{p}")
                          for p in range(4)]
                    for mc in range(32):
                        adj_t = wk.tile([128, S], BF16, tag="adj", bufs=2)
                        nc.sync.dma_start(out=adj_t[:], in_=adjt[mc])
                        vm_t = wk.tile([128, B * D], BF16, tag="vml", bufs=2)
                        lc = mc % 4
                        nc.sync.dma_start(
                            out=vm_t[:],
                            in_=vm_all[mc // 4, lc * 128:(lc + 1) * 128,
                                       :, :])
                        for p in range(4):
                            nc.tensor.matmul(
                                pg[p][:],
                                lhsT=vm_t[:, p * 128:(p + 1) * 128],
                                rhs=adj_t[:], start=(mc == 0),
                                stop=(mc == 31))
                    for p in range(4):
                        gt = wk.tile([128, S], F32, tag="gt", bufs=2)
                        nc.vector.tensor_mul(gt[:], pg[p][:], rrs_bc[:])
                        ot = wk.tile([128, S], F32, tag="ot", bufs=2)
                        nc.vector.tensor_add(ot[:], gt[:], attn_sb[p][:])
                        nc.sync.dma_start(out=out[2 * p], in_=ot[0:D, :])
                        nc.sync.dma_start(out=out[2 * p + 1], in_=ot[D:128, :])
    nc.compile()
    return nc


def _prep_inputs(query_input, source_input, adj, Wq_w, Wq_b, Wk_w, Wk_b,
                 Wv_w, Wv_b):
    bf = ml_dtypes.bfloat16
    xq_np = np.asarray(query_input, dtype=np.float32)
    xs_np = np.asarray(source_input, dtype=np.float32)
    adj_np = np.asarray(adj, dtype=np.float32)

    adjT = np.ascontiguousarray(adj_np.T)
    np.fill_diagonal(adjT, adjT.diagonal() + 1.0)
    adjT_bf = adjT.astype(bf)
    rrs_full = (0.25 / (adj_np.sum(axis=1) + 1.0)).astype(np.float32)

    wqt = np.ascontiguousarray(np.asarray(Wq_w, np.float32).T).reshape(
        2, 128, HD).astype(bf)
    wkt = np.ascontiguousarray(np.asarray(Wk_w, np.float32).T).reshape(
        2, 128, HD)
    wvt = np.ascontiguousarray(np.asarray(Wv_w, np.float32).T).reshape(
        2, 128, HD)
    wkv_np = np.concatenate([wkt, wvt], axis=2).astype(bf)
    bqc = np.ascontiguousarray(
        np.asarray(Wq_b, np.float32).reshape(2, 128).T)
    bkv_np = np.concatenate(
        [np.asarray(Wk_b, np.float32).reshape(1, HD),
         np.asarray(Wv_b, np.float32).reshape(1, HD)], axis=1).astype(bf)

    # batch-indicator tiles for [32,S]/[B,HD] accumulating matmuls
    iqa = np.zeros((128, B * 32), np.float32)
    iqb = np.zeros((128, B * 32), np.float32)
    iv8 = np.zeros((128, B * B), np.float32)
    for b in range(B):
        for p in range(128):
            iqa[p, b * 32 + 4 * b + p // 64] = 1.0
            iqb[p, b * 32 + 4 * b + 2 + p // 64] = 1.0
            iv8[p, b * B + b] = 1.0
    ibc0 = np.zeros((4, 128), np.float32)
    ibc1 = np.zeros((4, 128), np.float32)
    for p in range(128):
        ibc0[p // 64, p] = 1.0
        ibc1[2 + p // 64, p] = 1.0

    xq_bf = xq_np.astype(bf)
    xs_bf = xs_np.astype(bf)
    in_maps = []
    for i in range(NCORES):
        sl = slice(i * S, (i + 1) * S)
        in_maps.append({
            "xq": np.ascontiguousarray(xq_bf[:, :, sl]).reshape(B, 2, 128, S),
            "xs": np.ascontiguousarray(xs_bf[:, :, sl]).reshape(B, 2, 128, S),
            "adjt": np.ascontiguousarray(adjT_bf[:, sl]).reshape(32, 128, S),
            "rrs": np.ascontiguousarray(rrs_full[sl]).reshape(1, S),
            "wqt": wqt, "wkv": wkv_np,
            "bqc": bqc, "bkv": bkv_np,
            "iqa_in": iqa.astype(bf), "iqb_in": iqb.astype(bf),
            "iv8_in": iv8.astype(bf),
            "ibc0_in": ibc0.astype(bf), "ibc1_in": ibc1.astype(bf),
            "ones_rb": np.ones((1, 128), bf),
            "ones_rf": np.ones((1, 128), np.float32),
            "ones_c": np.ones((128, 1), bf),
        })
    return in_maps


def kernel(**inputs):
    if "nc" not in _CACHE:
        _CACHE["nc"] = _build()
    nc = _CACHE["nc"]
    in_maps = _prep_inputs(**inputs)
    res = run_bass_kernel_spmd(nc, in_maps, list(range(NCORES)))
    full = np.empty((B, D, N), np.float32)
    for i in range(NCORES):
        full[:, :, i * S:(i + 1) * S] = res.results[i]["out"]
    return full


# revision 14
# speedup vs baseline: 1.8060x; 1.0388x over previous
"""DIFFormerConv (simple linear attention + dense GCN) on 8 trn2 NeuronCores.

Sharding: nodes N=4096 split 8 ways (S=512 per core). Each core computes
q/k/v for its node shard, partial kvs/ks_sum/vsum (AllReduce), vmean
(AllGather, bf16), the attention output rows for its shard, and the GCN
rows for its shard (adj^T column shard, bf16 matmul).

All matmuls run in bf16 (full PE rate). Phase order maximizes
collective overlap: k/v/kvs/vmean for all batches -> AllReduce +
AllGather issued -> q projections run under the collectives -> attention
epilogue (batched denominators) -> GCN.

Engines can only address SBUF/PSUM at partition offsets 0/32/64, so all
per-batch [4,*] rows are produced via indicator matmuls accumulating
into batched [32,S]/[B,HD] PSUM tiles, and repacked to [4, B*S] layout
with small SBUF->SBUF DMAs where matmul operands need base partition 0.

Layouts (no PE transposes needed):
  q:   [hd, s]  (heads*dim on partitions)  -- lhsT = W^T chunks, bias via
                                              per-partition activation add
  k,v: [s, hd]  (transposed projection)    -- lhsT = x chunks, bias via
                                              K=1 ones matmul
  kvs AllReduce payload: [2, 128, B, 65] f32 (64 diag cols + ks column)
  gcn: [(b,d), n] directly                 -- lhsT = vmean[m,(b,d)],
                                              rhs = adjT[m,n], bf16
Host prep: adjT = adj.T + I (bf16), rrs = 0.25/(rowsum+1), W^T in bf16.
"""

import sys

sys.path.insert(0, "/opt/trn_rl_repo")

import numpy as np
import ml_dtypes

from concourse import bass, bacc, tile, mybir
from concourse.bass_utils import run_bass_kernel_spmd

B, C, N, H, D = 8, 256, 4096, 4, 64
NCORES = 8
S = N // NCORES          # 512 nodes per core
HD = H * D               # 256
F32 = mybir.dt.float32
F32R = mybir.dt.float32r
BF16 = mybir.dt.bfloat16
AX = mybir.AxisListType.X
ALU = mybir.AluOpType
ACTF = mybir.ActivationFunctionType
RG = [list(range(NCORES))]

_CACHE = {}


def _build():
    nc = bacc.Bacc("TRN2", target_bir_lowering=False, debug=False,
                   num_devices=NCORES)

    xq = nc.dram_tensor("xq", [B, 2, 128, S], BF16, kind="ExternalInput")
    xs = nc.dram_tensor("xs", [B, 2, 128, S], BF16, kind="ExternalInput")
    adjt = nc.dram_tensor("adjt", [32, 128, S], BF16, kind="ExternalInput")
    rrs = nc.dram_tensor("rrs", [1, S], F32R, kind="ExternalInput")
    wqt = nc.dram_tensor("wqt", [2, 128, HD], BF16, kind="ExternalInput")
    wkv = nc.dram_tensor("wkv", [2, 128, 2 * HD], BF16, kind="ExternalInput")
    bqc = nc.dram_tensor("bqc", [128, 2], F32, kind="ExternalInput")
    bkv = nc.dram_tensor("bkv", [1, 2 * HD], BF16, kind="ExternalInput")
    out = nc.dram_tensor("out", [B, D, S], F32, kind="ExternalOutput")

    # indicator tensors (see _prep_inputs)
    iqa_d = nc.dram_tensor("iqa_in", [128, B * 32], BF16, kind="ExternalInput")
    iqb_d = nc.dram_tensor("iqb_in", [128, B * 32], BF16, kind="ExternalInput")
    iv8_d = nc.dram_tensor("iv8_in", [128, B * B], BF16, kind="ExternalInput")
    ibc0_d = nc.dram_tensor("ibc0_in", [4, 128], BF16, kind="ExternalInput")
    ibc1_d = nc.dram_tensor("ibc1_in", [4, 128], BF16, kind="ExternalInput")
    ones_rb_d = nc.dram_tensor("ones_rb", [1, 128], BF16, kind="ExternalInput")
    ones_rf_d = nc.dram_tensor("ones_rf", [1, 128], F32R, kind="ExternalInput")

    with nc.allow_low_precision(reason="bf16 matmul pipeline intentional"), \
            tile.TileContext(nc) as tc:
        with (
            tc.tile_pool(name="pers", bufs=1) as pp,
            tc.tile_pool(name="work", bufs=3) as wk,
            tc.tile_pool(name="dram", bufs=1, space="DRAM") as dp,
        ):
            # DRAM internal buffers for collectives
            vm_loc = dp.tile([S, B, D], BF16, tag="vm_loc", name="vm_loc")
            vm_all = dp.tile([NCORES, S, B, D], BF16, tag="vm_all",
                             name="vm_all", addr_space="Shared")
            # single AllReduce payload (collectives have a large fixed
            # latency -- one op beats two): row h = [128, B*65] kvs+ks half
            # followed by 4 batches' vs rows. 2-D shape with a large inner
            # dim -- a flat 1-D tile lowers to a degenerate AP and the
            # collective runs ~3x slower.
            KVH = 128 * B * 65
            ARW = KVH + B * HD // 2
            arf_in = dp.tile([2, ARW], BF16, tag="arf_in", name="arf_in")
            arf_out = dp.tile([2, ARW], BF16, tag="arf_out",
                              name="arf_out", addr_space="Shared")

            def ar_kvs(t, h):
                return t[h, 0:KVH].rearrange("(p c) -> p c", p=128)

            def ar_vs(t, h):
                # batches 4h..4h+3
                return t[h, KVH:ARW].rearrange("(b c) -> b c", b=B // 2)

            # ---- constants ----
            # phase-A constants only -- everything else loads later, off
            # the startup critical path
            wkv_t = [pp.tile([128, 2 * HD], BF16, tag=f"wkv{c}",
                             name=f"wkv{c}") for c in range(2)]
            for c in range(2):
                nc.sync.dma_start(out=wkv_t[c][:], in_=wkv[c])
            bkv_sb = pp.tile([1, 2 * HD], BF16, tag="bkvsb")
            nc.sync.dma_start(out=bkv_sb[:], in_=bkv[:])
            ones_rb = pp.tile([1, 128], BF16, tag="ones_rb")
            nc.sync.dma_start(out=ones_rb[:], in_=ones_rb_d[:])
            iv8 = pp.tile([128, B * B], BF16, tag="iv8")
            nc.sync.dma_start(out=iv8[:], in_=iv8_d[:])
            wq_t = [pp.tile([128, HD], BF16, tag=f"wq{c}", name=f"wq{c}")
                    for c in range(2)]
            bq_sb = pp.tile([128, 2], F32, tag="bqsb")
            iqa = pp.tile([128, B * 32], BF16, tag="iqa")
            iqb = pp.tile([128, B * 32], BF16, tag="iqb")
            ibc0 = pp.tile([4, 128], BF16, tag="ibc0")
            ibc1 = pp.tile([4, 128], BF16, tag="ibc1")
            ones_rf = pp.tile([1, 128], F32R, tag="ones_rf")
            rrs_row = pp.tile([1, S], F32R, tag="rrs_row")

            # persistent SBUF tensors
            q_sb = [[pp.tile([128, S], BF16, tag=f"q{b}_{h}",
                             name=f"q{b}_{h}")
                     for h in range(2)] for b in range(B)]
            ss_all = pp.tile([32, S], F32, tag="ss_all")
            rq_all = pp.tile([32, S], F32, tag="rq_all")
            vmacc = [pp.tile([128, B * D], BF16, tag=f"vmacc{c}",
                             name=f"vmacc{c}")
                     for c in range(4)]
            attn_sb = [pp.tile([128, S], F32, tag=f"at{p}", name=f"at{p}")
                       for p in range(4)]
            rrs_bc = pp.tile([128, S], F32, tag="rrs_bc")

            with tc.tile_pool(name="psA", bufs=1, space="PSUM") as psA:
                # ============ phase A: k/v/kvs/vmean for all batches ========
                # merged k|v projection ([128,512] matmuls), then batched
                # normalization per batch so the PE never waits on the
                # scalar/vector normalize chain chunk-by-chunk
                vs8_ps = psA.tile([B, HD], F32, tag="vs8")
                VW = HD + 1  # vt chunk stride (col 256 of each chunk = ones)
                for b in range(B):
                    xs0 = wk.tile([128, S], BF16, tag="xs0", bufs=2)
                    xs1 = wk.tile([128, S], BF16, tag="xs1", bufs=2)
                    nc.sync.dma_start(out=xs0[:], in_=xs[b, 0])
                    nc.sync.dma_start(out=xs1[:], in_=xs[b, 1])

                    k_all = wk.tile([128, 4 * HD], BF16, tag="k_all", bufs=2)
                    vt_all = wk.tile([128, 4 * VW], BF16, tag="vt_all",
                                     bufs=2)
                    for c4 in range(4):
                        sl = slice(c4 * 128, (c4 + 1) * 128)
                        pkv = psA.tile([128, 2 * HD], F32, tag="pkv",
                                       bufs=2)
                        nc.tensor.matmul(pkv[:], lhsT=xs0[:, sl],
                                         rhs=wkv_t[0][:], start=True,
                                         stop=False)
                        nc.tensor.matmul(pkv[:], lhsT=xs1[:, sl],
                                         rhs=wkv_t[1][:], start=False,
                                         stop=False)
                        nc.tensor.matmul(pkv[:], lhsT=ones_rb[:],
                                         rhs=bkv_sb[:], start=False,
                                         stop=True)
                        nc.scalar.activation(
                            k_all[:, c4 * HD:(c4 + 1) * HD], pkv[:, 0:HD],
                            ACTF.Copy)
                        nc.vector.tensor_scalar_mul(
                            vt_all[:, c4 * VW:c4 * VW + HD],
                            pkv[:, HD:2 * HD], 1.0)
                        nc.vector.memset(
                            vt_all[:, c4 * VW + HD:c4 * VW + HD + 1], 1.0)
                        nc.vector.reduce_sum(
                            vmacc[c4][:, b * D:(b + 1) * D],
                            pkv[:, HD:2 * HD].rearrange("p (h d) -> p d h",
                                                        h=H), axis=AX)
                    # batched normalization for all 4 chunks x 4 heads
                    sq = wk.tile([128, 4 * HD], F32, tag="sq", bufs=2)
                    nc.scalar.activation(sq[:], k_all[:], ACTF.Square)
                    ssk = wk.tile([128, 16], F32, tag="ssk", bufs=2)
                    nc.vector.reduce_sum(
                        ssk[:], sq[:].rearrange("p (g d) -> p g d", g=16),
                        axis=AX)
                    snk = wk.tile([128, 16], F32, tag="snk", bufs=2)
                    nc.scalar.activation(snk[:], ssk[:], ACTF.Sqrt)
                    rk = wk.tile([128, 16], F32, tag="rk", bufs=2)
                    nc.vector.reciprocal(rk[:], snk[:])
                    kt_all = wk.tile([128, 4 * HD], BF16, tag="kt_all",
                                     bufs=2)
                    for g in range(16):
                        gs = slice(g * D, (g + 1) * D)
                        nc.vector.tensor_scalar_mul(
                            kt_all[:, gs], k_all[:, gs], rk[:, g:g + 1])

                    # kvs+ks / vs accumulation (one PSUM bank per half --
                    # interleaved groups within one bank lose the first
                    # group's start contribution)
                    kvs_e0 = psA.tile([128, HD + 1], F32, tag="kvse0")
                    kvs_e1 = psA.tile([128, HD + 1], F32, tag="kvse1")
                    for c4 in range(4):
                        st = c4 == 0
                        sp = c4 == 3
                        vchunk = vt_all[:, c4 * VW:c4 * VW + HD + 1]
                        nc.tensor.matmul(kvs_e0[:],
                                         lhsT=kt_all[:, c4 * HD:c4 * HD + 128],
                                         rhs=vchunk, start=st, stop=sp)
                        nc.tensor.matmul(kvs_e1[:],
                                         lhsT=kt_all[:,
                                                     c4 * HD + 128:c4 * HD + HD],
                                         rhs=vchunk, start=st, stop=sp)
                        nc.tensor.matmul(vs8_ps[:],
                                         lhsT=iv8[:, b * B:(b + 1) * B],
                                         rhs=vt_all[:, c4 * VW:c4 * VW + HD],
                                         start=(b == 0 and st),
                                         stop=(b == B - 1 and sp))

                    # evac kvs diag blocks + ks column -> AR1 payload (bf16)
                    pk0 = wk.tile([128, 65], BF16, tag="arpk0", bufs=2)
                    pk1 = wk.tile([128, 65], BF16, tag="arpk1", bufs=2)
                    nc.scalar.activation(pk0[0:64, 0:64], kvs_e0[0:64, 0:64],
                                         ACTF.Copy)
                    nc.scalar.activation(pk0[64:128, 0:64],
                                         kvs_e0[64:128, 64:128], ACTF.Copy)
                    nc.scalar.activation(pk0[:, 64:65], kvs_e0[:, HD:HD + 1],
                                         ACTF.Copy)
                    nc.scalar.activation(pk1[0:64, 0:64],
                                         kvs_e1[0:64, 128:192], ACTF.Copy)
                    nc.scalar.activation(pk1[64:128, 0:64],
                                         kvs_e1[64:128, 192:256], ACTF.Copy)
                    nc.scalar.activation(pk1[:, 64:65], kvs_e1[:, HD:HD + 1],
                                         ACTF.Copy)
                    nc.sync.dma_start(
                        out=ar_kvs(arf_in, 0)[:, b * 65:(b + 1) * 65],
                        in_=pk0[:])
                    nc.sync.dma_start(
                        out=ar_kvs(arf_in, 1)[:, b * 65:(b + 1) * 65],
                        in_=pk1[:])

                # vs rows + vmean chunks to DRAM
                vs8_sb = wk.tile([B, HD], BF16, tag="vs8sb", bufs=1)
                nc.scalar.activation(vs8_sb[:], vs8_ps[:], ACTF.Copy)
                nc.sync.dma_start(out=ar_vs(arf_in, 0), in_=vs8_sb[0:4, :])
                nc.sync.dma_start(out=ar_vs(arf_in, 1), in_=vs8_sb[4:8, :])
                for c in range(4):
                    nc.sync.dma_start(
                        out=vm_loc[c * 128:(c + 1) * 128, :, :],
                        in_=vmacc[c][:])

                # ================= collectives (overlap with q work) ========
                nc.gpsimd.collective_compute(
                    "AllReduce", ALU.add, ins=[arf_in.opt()],
                    outs=[arf_out.opt()], replica_groups=RG)
                nc.gpsimd.collective_compute(
                    "AllGather", ALU.bypass, ins=[vm_loc.opt()],
                    outs=[vm_all.opt()], replica_groups=RG)

                # ============ phase B: q projections (under collectives) ====
                # late constants (startup path only needs phase-A tensors)
                for c in range(2):
                    nc.sync.dma_start(out=wq_t[c][:], in_=wqt[c])
                nc.sync.dma_start(out=bq_sb[:], in_=bqc[:])
                nc.sync.dma_start(out=iqa[:], in_=iqa_d[:])
                nc.sync.dma_start(out=iqb[:], in_=iqb_d[:])
                nc.sync.dma_start(out=ibc0[:], in_=ibc0_d[:])
                nc.sync.dma_start(out=ibc1[:], in_=ibc1_d[:])
                nc.sync.dma_start(out=ones_rf[:], in_=ones_rf_d[:])
                nc.sync.dma_start(out=rrs_row[:], in_=rrs[:])
                ss32_ps = psA.tile([32, S], F32, tag="ss32")
                for b in range(B):
                    xq0 = wk.tile([128, S], BF16, tag="xs0", bufs=2)
                    xq1 = wk.tile([128, S], BF16, tag="xs1", bufs=2)
                    nc.sync.dma_start(out=xq0[:], in_=xq[b, 0])
                    nc.sync.dma_start(out=xq1[:], in_=xq[b, 1])
                    for h in range(2):
                        hsl = slice(h * 128, (h + 1) * 128)
                        pq = psA.tile([128, S], F32, tag="pq", bufs=2)
                        nc.tensor.matmul(pq[:], lhsT=wq_t[0][:, hsl],
                                         rhs=xq0[:], start=True, stop=False)
                        nc.tensor.matmul(pq[:], lhsT=wq_t[1][:, hsl],
                                         rhs=xq1[:], start=False, stop=True)
                        nc.vector.tensor_scalar(
                            q_sb[b][h][:], pq[:], bq_sb[:, h:h + 1], None,
                            op0=ALU.add)
                        qsq = wk.tile([128, S], BF16, tag="qsq", bufs=2)
                        nc.scalar.activation(qsq[:], pq[:], ACTF.Square,
                                             bias=bq_sb[:, h:h + 1])
                        # |q|^2 rows land at [4b..4b+3] via batch indicator
                        nc.tensor.matmul(
                            ss32_ps[:],
                            lhsT=(iqa if h == 0 else iqb)[:,
                                                          b * 32:(b + 1) * 32],
                            rhs=qsq[:], start=(b == 0 and h == 0),
                            stop=(b == B - 1 and h == 1))
                # rq = 1/|q| for all batches in one op
                nc.scalar.activation(ss_all[:], ss32_ps[:], ACTF.Sqrt)
                nc.vector.reciprocal_approx_fast(rq_all[:], ss_all[:])
                # broadcast rrs row to all 128 partitions (K=1 matmul)
                pbc0 = psA.tile([128, S], F32, tag="pq", bufs=2)
                nc.tensor.matmul(pbc0[:], lhsT=ones_rf[:], rhs=rrs_row[:],
                                 start=True, stop=True)
                nc.scalar.activation(rrs_bc[:], pbc0[:], ACTF.Copy)

            # ===== phase C1: AR results, denominators =====================
            with tc.tile_pool(name="psB", bufs=2, space="PSUM") as psB:
                kpk_bf = []
                for h2 in range(2):
                    kf = wk.tile([128, B * 65], BF16, tag=f"kf{h2}", bufs=1,
                                 name=f"kf{h2}")
                    nc.sync.dma_start(out=kf[:], in_=ar_kvs(arf_out, h2))
                    kpk_bf.append(kf)
                vs8b = wk.tile([B, HD], BF16, tag="vs8b", bufs=1)
                nc.sync.dma_start(out=vs8b[0:4, :], in_=ar_vs(arf_out, 0))
                nc.sync.dma_start(out=vs8b[4:8, :], in_=ar_vs(arf_out, 1))
                # vs rows repacked to [4, B*64] (lhsT needs base partition 0)
                vs4 = wk.tile([4, B * D], BF16, tag="vs4", bufs=1)
                for b in range(B):
                    nc.sync.dma_start(
                        out=vs4[:, b * D:(b + 1) * D],
                        in_=vs8b[b:b + 1, :])

                # masked ks tiles: block b col (4b+2*h2+hh) = head hh of
                # half h2 for batch b (zeros elsewhere)
                ksm = []
                for h2 in range(2):
                    km = wk.tile([128, 32 * B], BF16, tag=f"ksm{h2}", bufs=1,
                                 name=f"ksm{h2}")
                    nc.vector.memset(km[:], 0.0)
                    for b in range(B):
                        col = b * 65 + 64
                        for hh in range(2):
                            rs = slice(hh * 64, hh * 64 + 64)
                            dst = b * 32 + 4 * b + 2 * h2 + hh
                            nc.scalar.activation(
                                km[rs, dst:dst + 1],
                                kpk_bf[h2][rs, col:col + 1], ACTF.Copy)
                    ksm.append(km)
                # denominators: pden32[4b+j] = q_j . ks_j (accumulated)
                pden32 = psB.tile([32, S], F32, tag="pden", bufs=1)
                for b in range(B):
                    nc.tensor.matmul(pden32[:],
                                     lhsT=ksm[0][:, b * 32:(b + 1) * 32],
                                     rhs=q_sb[b][0][:], start=(b == 0),
                                     stop=False)
                    nc.tensor.matmul(pden32[:],
                                     lhsT=ksm[1][:, b * 32:(b + 1) * 32],
                                     rhs=q_sb[b][1][:], start=False,
                                     stop=(b == B - 1))
                t0 = wk.tile([32, S], F32, tag="t0", bufs=1)
                nc.vector.tensor_mul(t0[:], pden32[:], rq_all[:])
                t1 = wk.tile([32, S], F32, tag="t1", bufs=1)
                nc.vector.tensor_scalar(t1[:], t0[:], 4.0, float(4 * N),
                                        op0=ALU.mult, op1=ALU.add)
                rp_f = wk.tile([32, S], F32, tag="rpf", bufs=1)
                nc.vector.reciprocal_approx_fast(rp_f[:], t1[:])
                cc_f = wk.tile([32, S], F32, tag="ccf", bufs=1)
                nc.vector.tensor_mul(cc_f[:], rp_f[:], rq_all[:])
                rp_b32 = wk.tile([32, S], BF16, tag="rpb32", bufs=1)
                cc_b32 = wk.tile([32, S], BF16, tag="ccb32", bufs=1)
                nc.scalar.activation(rp_b32[:], rp_f[:], ACTF.Copy)
                nc.scalar.activation(cc_b32[:], cc_f[:], ACTF.Copy)
                # repack to [4, B*S] via SBUF->SBUF DMA (base partition 0)
                rp4 = wk.tile([4, B * S], BF16, tag="rp4", bufs=1)
                cc4 = wk.tile([4, B * S], BF16, tag="cc4", bufs=1)
                for b in range(B):
                    cs = slice(b * S, (b + 1) * S)
                    nc.sync.dma_start(out=rp4[:, cs],
                                      in_=rp_b32[4 * b:4 * b + 4, :])
                    nc.sync.dma_start(out=cc4[:, cs],
                                      in_=cc_b32[4 * b:4 * b + 4, :])

                # ===== phase C2: per-head cc broadcast, qs = qn * cc ========
                qs_all = [[pp.tile([128, S], BF16, tag=f"qs{b}_{h}",
                                   name=f"qs{b}_{h}") for h in range(2)]
                          for b in range(B)]
                for b in range(B):
                    for h2 in range(2):
                        pbc = psB.tile([128, S], F32, tag="pbc")
                        nc.tensor.matmul(
                            pbc[:],
                            lhsT=(ibc0[:] if h2 == 0 else ibc1[:]),
                            rhs=cc4[:, b * S:(b + 1) * S],
                            start=True, stop=True)
                        nc.vector.tensor_mul(qs_all[b][h2][:],
                                             q_sb[b][h2][:], pbc[:])

                # ===== phase D: GCN matmuls (AllGather-gated) ===============
                with tc.tile_pool(name="psC", bufs=1, space="PSUM") as psC:
                    pg = [psC.tile([128, S], F32, tag=f"g{p}", name=f"g{p}")
                          for p in range(4)]
                    for mc in range(32):
                        adj_t = wk.tile([128, S], BF16, tag="adj", bufs=2)
                        nc.sync.dma_start(out=adj_t[:], in_=adjt[mc])
                        vm_t = wk.tile([128, B * D], BF16, tag="vml", bufs=2)
                        lc = mc % 4
                        nc.sync.dma_start(
                            out=vm_t[:],
                            in_=vm_all[mc // 4, lc * 128:(lc + 1) * 128,
                                       :, :])
                        for p in range(4):
                            nc.tensor.matmul(
                                pg[p][:],
                                lhsT=vm_t[:, p * 128:(p + 1) * 128],
                                rhs=adj_t[:], start=(mc == 0),
                                stop=(mc == 31))

                    # ===== phase C3: attention output (pure PE chain) =======
                    for b in range(B):
                        pat = psB.tile([D, S], F32, tag="pat", bufs=1)
                        for h2 in range(2):
                            nc.tensor.matmul(
                                pat[:],
                                lhsT=kpk_bf[h2][:, b * 65:b * 65 + 64],
                                rhs=qs_all[b][h2][:], start=(h2 == 0),
                                stop=False)
                        nc.tensor.matmul(
                            pat[:], lhsT=vs4[:, b * D:(b + 1) * D],
                            rhs=rp4[:, b * S:(b + 1) * S],
                            start=False, stop=True)
                        nc.scalar.activation(
                            attn_sb[b // 2][(b % 2) * D:(b % 2 + 1) * D, :],
                            pat[:], ACTF.Copy)

                    # ===== final: gcn * rrs + attn, store ===================
                    for p in range(4):
                        gt = wk.tile([128, S], F32, tag="gt", bufs=2)
                        nc.vector.tensor_mul(gt[:], pg[p][:], rrs_bc[:])
                        ot = wk.tile([128, S], F32, tag="ot", bufs=2)
                        nc.vector.tensor_add(ot[:], gt[:], attn_sb[p][:])
                        nc.sync.dma_start(out=out[2 * p], in_=ot[0:D, :])
                        nc.sync.dma_start(out=out[2 * p + 1], in_=ot[D:128, :])
    nc.compile()
    return nc


def _prep_inputs(query_input, source_input, adj, Wq_w, Wq_b, Wk_w, Wk_b,
                 Wv_w, Wv_b):
    bf = ml_dtypes.bfloat16
    xq_np = np.asarray(query_input, dtype=np.float32)
    xs_np = np.asarray(source_input, dtype=np.float32)
    adj_np = np.asarray(adj, dtype=np.float32)

    adjT = np.ascontiguousarray(adj_np.T)
    np.fill_diagonal(adjT, adjT.diagonal() + 1.0)
    adjT_bf = adjT.astype(bf)
    rrs_full = (0.25 / (adj_np.sum(axis=1) + 1.0)).astype(np.float32)

    wqt = np.ascontiguousarray(np.asarray(Wq_w, np.float32).T).reshape(
        2, 128, HD).astype(bf)
    wkt = np.ascontiguousarray(np.asarray(Wk_w, np.float32).T).reshape(
        2, 128, HD)
    wvt = np.ascontiguousarray(np.asarray(Wv_w, np.float32).T).reshape(
        2, 128, HD)
    wkv_np = np.concatenate([wkt, wvt], axis=2).astype(bf)
    bqc = np.ascontiguousarray(
        np.asarray(Wq_b, np.float32).reshape(2, 128).T)
    bkv_np = np.concatenate(
        [np.asarray(Wk_b, np.float32).reshape(1, HD),
         np.asarray(Wv_b, np.float32).reshape(1, HD)], axis=1).astype(bf)

    # batch-indicator tiles for [32,S]/[B,HD] accumulating matmuls
    iqa = np.zeros((128, B * 32), np.float32)
    iqb = np.zeros((128, B * 32), np.float32)
    iv8 = np.zeros((128, B * B), np.float32)
    for b in range(B):
        for p in range(128):
            iqa[p, b * 32 + 4 * b + p // 64] = 1.0
            iqb[p, b * 32 + 4 * b + 2 + p // 64] = 1.0
            iv8[p, b * B + b] = 1.0
    ibc0 = np.zeros((4, 128), np.float32)
    ibc1 = np.zeros((4, 128), np.float32)
    for p in range(128):
        ibc0[p // 64, p] = 1.0
        ibc1[2 + p // 64, p] = 1.0

    xq_bf = xq_np.astype(bf)
    xs_bf = xs_np.astype(bf)
    in_maps = []
    for i in range(NCORES):
        sl = slice(i * S, (i + 1) * S)
        in_maps.append({
            "xq": np.ascontiguousarray(xq_bf[:, :, sl]).reshape(B, 2, 128, S),
            "xs": np.ascontiguousarray(xs_bf[:, :, sl]).reshape(B, 2, 128, S),
            "adjt": np.ascontiguousarray(adjT_bf[:, sl]).reshape(32, 128, S),
            "rrs": np.ascontiguousarray(rrs_full[sl]).reshape(1, S),
            "wqt": wqt, "wkv": wkv_np,
            "bqc": bqc, "bkv": bkv_np,
            "iqa_in": iqa.astype(bf), "iqb_in": iqb.astype(bf),
            "iv8_in": iv8.astype(bf),
            "ibc0_in": ibc0.astype(bf), "ibc1_in": ibc1.astype(bf),
            "ones_rb": np.ones((1, 128), bf),
            "ones_rf": np.ones((1, 128), np.float32),
        })
    return in_maps


def kernel(**inputs):
    if "nc" not in _CACHE:
        _CACHE["nc"] = _build()
    nc = _CACHE["nc"]
    in_maps = _prep_inputs(**inputs)
    res = run_bass_kernel_spmd(nc, in_maps, list(range(NCORES)))
    full = np.empty((B, D, N), np.float32)
    for i in range(NCORES):
        full[:, :, i * S:(i + 1) * S] = res.results[i]["out"]
    return full
